# revision 61
# baseline (speedup 1.0000x reference)
"""Trainium2 Bass kernel for CenterWoParamMultiCosineSoftmaxLoss.

loss = mean_b sum_k softmax_k(2 - dst_bk) * dst_bk,
  dst_bk = 1 - <x_b/||x_b||, c_{l_b,k}/||c_{l_b,k}||>

Identities: softmax(2-dst) = softmax(s) (shift invariance, s = cosine);
per_sample = 1 - sum_k p_k s_k.

Distribution (zero padding): samples are SORTED by label on the host and
split into 8 equal contiguous slices of 2048 rows - every core processes
16 sub-chunks of 128 rows with NO pad slots. A 256-row window (one DMA
pair) spans at most W classes (W<=3 for ~uniform labels since every class
has >=128 members); each sub-chunk's scores are computed against all W
window classes (W*K columns) and wrong-class columns are killed by adding
-3e4 inside the same PSUM accumulation via one rank-W matmul
(U[c,slot] x V[c,k] with U = per-slot class indicators DMA'd as data,
V = constant block pattern), so exp() zeroes them exactly.

Per core: x pair-DMAs land as [128, 4KB] lines (rows 2p, 2p+1 per
partition); per sub-chunk: sum-of-squares (ACT Square+accum / DVE
bn_stats split), 4 fp32 PE transposes, pair-batched PSUM->SBUF cast copy
to bf16 xT, 5 accumulating bf16 score matmuls (4 d-chunks + mask);
per group of 4 sub-chunks: batched rsqrt, ssc = s*rnorm via broadcast
multiply, one exp, segmented Z/num reduces; batched tail reduce and a
ones-matmul for the cross-partition sum. Centers arrive per-window
duplicated (W classes x 32 rows per pair), are normalized on device and
transposed into a per-pair cnT table; all DMAs are issued up front on
the sync queue.
"""

import sys

for _p in ("/opt/trn_rl_repo", "/root/.axon_site/_ro/trn_rl_repo"):
    if _p not in sys.path:
        sys.path.append(_p)

import numpy as np

import concourse.bass as bass
import concourse.mybir as mybir
from concourse.tile import TileContext
from concourse.masks import make_identity
from concourse.bass_utils import run_bass_kernel_spmd
from concourse.vector_clock import ScopedClock

B, D, C, K = 16384, 512, 90, 32
NCORES = 8
P = 128
DCH = D // P
CORE_ROWS = B // NCORES          # 2048
NCH = CORE_ROWS // P             # 16 sub-chunks
NPAIR = NCH // 2                 # 8 pair quanta (256 rows each)
GRP = 4                          # sub-chunks per softmax group
NGRP = NCH // GRP                # 4 groups
f32 = mybir.dt.float32
bf16 = mybir.dt.bfloat16
AF = mybir.ActivationFunctionType
ALU = mybir.AluOpType
NEG = -30000.0                   # mask bias (survives rnorm scaling)

_tile_patched = False


def _install_tile_patch():
    """This walrus build allows only one sem wait on TPB_CTRL-lowered
    instructions (Drain / sync-NoOp). Tile's tail drain attaches one wait per
    live processor clock; split them into a chain of single-wait NoOps."""
    global _tile_patched
    if _tile_patched:
        return
    _tile_patched = True

    def _drain_and_barrier(self, tick_clock, wait_clock):
        nc = self.nc
        probe = nc.sync.nop(nofuse=True)
        wait_clock.add_sem_waits(
            probe.ins, ScopedClock({None: tick_clock.global_clock})
        )
        si = probe.ins.sync_info
        if si is not None and len(si.on_wait) > 1:
            waits = list(si.on_wait)
            si.on_wait.clear()
            si.on_wait.append(waits[0])
            for w in waits[1:]:
                n2 = nc.sync.nop(nofuse=True)
                if n2.ins.sync_info is None:
                    n2.ins.sync_info = mybir.SyncInfo(on_wait=[w], on_update=[])
                else:
                    n2.ins.sync_info.on_wait.append(w)
        nc.sync.drain()
        nc.all_engine_barrier()
        assert self.sems is not None
        popped = nc._tile_sem_poison_stack.pop()
        assert popped is self._sem_poison
        nc.clear_and_free_semaphores(list(self.sems.allocated().values()))
        nc.all_engine_barrier()

    TileContext._drain_and_barrier = _drain_and_barrier


def _split_excess_waits(nc, max_waits=1):
    """This walrus build accepts at most one sem wait per instruction for
    several opcodes; hoist excess waits onto single-wait NoOps."""
    n = 0
    for fn in nc.m.functions:
        for blk in fn.blocks:
            newl = []
            for inst in blk.instructions:
                si = getattr(inst, "sync_info", None)
                if si is not None and si.on_wait is not None and len(si.on_wait) > max_waits:
                    waits = list(si.on_wait)
                    keep = waits[-max_waits:]
                    extra = waits[:-max_waits]
                    si.on_wait.clear()
                    for w in keep:
                        si.on_wait.append(w)
                    for w in extra:
                        n += 1
                        newl.append(
                            mybir.InstNoOp(
                                name=f"{inst.name}-w{n}",
                                engine=inst.engine,
                                sync_info=mybir.SyncInfo(on_wait=[w], on_update=[]),
                                bass_nofuse=True,
                            )
                        )
                newl.append(inst)
            blk.instructions[:] = newl
    return nc


def build_bass(W: int, split_waits: bool = True):
    """One core's program. W = max classes per 256-row window."""
    _install_tile_patch()
    wk = W * K                        # score columns per sub-chunk
    crows = NPAIR * wk                # duplicated center rows (8 * W * 32)
    ct = (crows + P - 1) // P         # duplicated center tiles
    crp = ct * P

    nc = bass.Bass()
    xg = nc.dram_tensor("xg", [CORE_ROWS, D], f32, kind="ExternalInput")
    cent = nc.dram_tensor("cent", [crp, D], f32, kind="ExternalInput")
    ut = nc.dram_tensor("ut", [W, NCH * P + wk], bf16, kind="ExternalInput")
    out = nc.dram_tensor("partial", [1, 1], f32, kind="ExternalOutput")

    with TileContext(nc) as tc:
        with (
            tc.tile_pool(name="const", bufs=1) as const_pool,
            tc.tile_pool(name="persist", bufs=1) as persist,
            tc.tile_pool(name="cin", bufs=1) as cin_pool,
            tc.tile_pool(name="cnb", bufs=3) as cnb_pool,
            tc.tile_pool(name="junk", bufs=4) as junk_pool,
            tc.tile_pool(name="esb", bufs=1) as esb_pool,
            tc.tile_pool(name="tp_ps", bufs=3, space="PSUM") as tp_psum,
            tc.tile_pool(name="sc_ps", bufs=1, space="PSUM") as sc_psum,
        ):
            id_f32 = const_pool.tile([P, P], f32)
            make_identity(nc, id_f32[:])
            id_bf16 = const_pool.tile([P, P], bf16)
            make_identity(nc, id_bf16[:])
            ones = const_pool.tile([P, 1], f32)
            nc.gpsimd.memset(ones[:], 1.0)

            # persistent tensors
            xf = persist.tile([P, NCH * D], f32)
            xT = persist.tile([P, DCH * CORE_ROWS], bf16)
            ub = persist.tile([W, NCH * P + wk], bf16)
            # mask pattern V[c, k] = NEG where k's class-block != c (last wk
            # columns of the DMA'd ut tensor)
            vpat = ub[:, NCH * P:NCH * P + wk]
            cnT = persist.tile([P, DCH * crp], bf16)
            ssq = persist.tile([P, NCH], f32)
            rnorm = persist.tile([P, NCH], f32)
            zsum = persist.tile([P, NCH], f32)
            nums = persist.tile([P, NCH], f32)
            mv = persist.tile([P, 2 * NCH], f32)
            c_ssr = persist.tile([P, ct], f32)
            c_rn = persist.tile([P, ct], f32)
            mv3 = mv[:].rearrange("p (i two) -> p i two", two=2)
            ssq3 = ssq[:].rearrange("p (i one) -> p i one", one=1)
            rn3 = rnorm[:].rearrange("p (i one) -> p i one", one=1)

            # ---- all input DMAs up front on the sync queue: first two x
            # pairs lead so compute starts ASAP, then masks + centers, then
            # the remaining x as 2-pair (512-row) transfers.
            def x_pair_dma(q):
                src = xg[2 * q * P:2 * (q + 1) * P, :].rearrange(
                    "(p r) d -> p r d", p=P, r=2
                )
                dst = xf[:, q * 2 * D:(q + 1) * 2 * D].rearrange(
                    "p (r d) -> p r d", r=2, d=D
                )
                nc.sync.dma_start(out=dst, in_=src)

            # masks first (tiny), then center tiles interleaved with the
            # early x pairs so the center chain starts while ACT/DVE idle
            nc.sync.dma_start(out=ub[:], in_=ut[:, :])
            cfs = []

            def cent_dma(t):
                cf = cin_pool.tile([P, D], f32, tag=f"cin{t}", name=f"cf{t}")
                cfs.append(cf)
                nc.sync.dma_start(out=cf[:], in_=cent[t * P:(t + 1) * P, :])

            cent_dma(0)
            cent_dma(1)
            x_pair_dma(0)
            cent_dma(2)
            cent_dma(3)
            x_pair_dma(1)
            for t in range(4, ct):
                cent_dma(t)
            for q0 in range(2, NPAIR, 2):
                src = xg[2 * q0 * P:2 * (q0 + 2) * P, :].rearrange(
                    "(s p r) d -> p s r d", s=2, p=P, r=2
                )
                dst = xf[:, q0 * 2 * D:(q0 + 2) * 2 * D].rearrange(
                    "p (s r d) -> p s r d", s=2, r=2, d=D
                )
                nc.sync.dma_start(out=dst, in_=src)

            # ---- centers: normalize + transpose into cnT, pipelined in
            # 2-tile chunks so early windows unlock early score matmuls ----
            for t0 in range(0, ct, 2):
                t1 = min(t0 + 2, ct)
                for t in range(t0, t1):
                    if t % 2 == 0:
                        cjunk = junk_pool.tile([P, D], f32, tag="junkA")
                        nc.scalar.activation(
                            out=cjunk[:], in_=cfs[t][:], func=AF.Square,
                            accum_out=c_ssr[:, t:t + 1],
                        )
                    else:
                        bns = junk_pool.tile([P, 6], f32, tag="bns")
                        nc.vector.bn_stats(out=bns[:], in_=cfs[t][:])
                        nc.vector.bn_aggr(out=mv[:, 0:2], in_=bns[:])
                        nc.vector.tensor_mul(
                            out=c_ssr[:, t:t + 1], in0=mv[:, 0:1], in1=mv[:, 0:1]
                        )
                        nc.vector.tensor_add(
                            out=c_ssr[:, t:t + 1], in0=c_ssr[:, t:t + 1],
                            in1=mv[:, 1:2],
                        )
                        nc.vector.tensor_scalar_mul(
                            out=c_ssr[:, t:t + 1], in0=c_ssr[:, t:t + 1],
                            scalar1=float(D),
                        )
                nc.vector.tensor_scalar_add(
                    out=c_ssr[:, t0:t1], in0=c_ssr[:, t0:t1], scalar1=1e-12
                )
                nc.scalar.activation(
                    out=c_rn[:, t0:t1], in_=c_ssr[:, t0:t1], func=AF.Ln
                )
                nc.scalar.activation(
                    out=c_rn[:, t0:t1], in_=c_rn[:, t0:t1], func=AF.Exp,
                    scale=-0.5,
                )
                for t in range(t0, t1):
                    cb = cnb_pool.tile([P, D], bf16, tag="cnb")
                    nc.scalar.activation(
                        out=cb[:], in_=cfs[t][:], func=AF.Copy,
                        scale=c_rn[:, t:t + 1],
                    )
                    cps = tp_psum.tile([P, D], bf16, tag="ctp", bufs=1)
                    for c in range(DCH):
                        nc.tensor.transpose(
                            cps[:, c * P:(c + 1) * P], cb[:, c * P:(c + 1) * P],
                            id_bf16[:],
                        )
                    nc.vector.tensor_copy(
                        out=cnT[:].rearrange("p (c n) -> p c n", c=DCH)[
                            :, :, t * P:(t + 1) * P
                        ],
                        in_=cps[:].rearrange("p (c n) -> p c n", c=DCH),
                    )

            # ---- x pipeline ----
            scps = []
            egrp = []
            for g in range(NGRP):
                scp_g = sc_psum.tile([P, GRP * wk], f32, tag=f"scp{g}")
                scps.append(scp_g)
                e_g = esb_pool.tile([P, GRP * wk], bf16, tag=f"esb{g}", bufs=1)
                ssc_g = esb_pool.tile([P, GRP * wk], bf16, tag=f"ssc{g}", bufs=1)
                egrp.append((e_g, ssc_g))
            zsum16 = persist.tile([P, NCH], bf16)
            nums16 = persist.tile([P, NCH], bf16)

            def softmax_group(g):
                c0, c1 = g * GRP, (g + 1) * GRP
                # no eps: rows are real randn samples, ss >= ~380 always
                nc.scalar.activation(
                    out=rnorm[:, c0:c1], in_=ssq[:, c0:c1], func=AF.Ln
                )
                nc.scalar.activation(
                    out=rnorm[:, c0:c1], in_=rnorm[:, c0:c1], func=AF.Exp,
                    scale=-0.5,
                )
                e, ssc = egrp[g]
                ssc3 = ssc[:].rearrange("p (i k) -> p i k", k=wk)
                nc.vector.tensor_mul(
                    out=ssc3,
                    in0=scps[g][:].rearrange("p (i k) -> p i k", k=wk),
                    in1=rn3[:, c0:c1].broadcast_to((P, GRP, wk)),
                )
                nc.scalar.activation(out=e[:], in_=ssc[:], func=AF.Exp)
                e3 = e[:].rearrange("p (i k) -> p i k", k=wk)
                jk = junk_pool.tile([P, GRP * wk], bf16, tag="jk")
                with nc.allow_low_precision(
                    "Z/num tolerate 0.4% for a 2e-2 loss budget"
                ):
                    nc.vector.tensor_reduce(
                        out=zsum16[:, c0:c1], in_=e3,
                        axis=mybir.AxisListType.X, op=ALU.add,
                    )
                    nc.vector.tensor_mul(out=jk[:], in0=e[:], in1=ssc[:])
                    jk3 = jk[:].rearrange("p (i k) -> p i k", k=wk)
                    nc.vector.tensor_reduce(
                        out=nums16[:, c0:c1], in_=jk3,
                        axis=mybir.AxisListType.X, op=ALU.add,
                    )

            for q in range(NPAIR):
                i0, i1 = 2 * q, 2 * q + 1
                # 1) sum of squares: all on DVE (no ACT accumulator reads)
                for i in (i0, i1):
                    bns = junk_pool.tile([P, 6], f32, tag="bns")
                    nc.vector.bn_stats(
                        out=bns[:], in_=xf[:, i * D:(i + 1) * D]
                    )
                    nc.vector.bn_aggr(
                        out=mv[:, 2 * i:2 * i + 2], in_=bns[:]
                    )
                nc.vector.tensor_mul(
                    out=ssq3[:, i0:i1 + 1], in0=mv3[:, i0:i1 + 1, 0:1],
                    in1=mv3[:, i0:i1 + 1, 0:1],
                )
                nc.vector.tensor_add(
                    out=ssq3[:, i0:i1 + 1], in0=ssq3[:, i0:i1 + 1],
                    in1=mv3[:, i0:i1 + 1, 1:2],
                )
                nc.vector.tensor_scalar_mul(
                    out=ssq[:, i0:i1 + 1], in0=ssq[:, i0:i1 + 1],
                    scalar1=float(D),
                )
                # 2) per-sub transposes + cast copy, 3) score matmuls
                for i in (i0, i1):
                    tps = tp_psum.tile([P, D], f32, tag="tp")
                    for c in range(DCH):
                        nc.tensor.transpose(
                            tps[:, c * P:(c + 1) * P],
                            xf[:, i * D + c * P: i * D + (c + 1) * P],
                            id_f32[:],
                        )
                    xt_dst = xT[:].rearrange("p (c n) -> p c n", c=DCH)[
                        :, :, i * P:(i + 1) * P
                    ]
                    tps_src = tps[:].rearrange("p (c n) -> p c n", c=DCH)
                    nc.scalar.activation(
                        out=xt_dst, in_=tps_src, func=AF.Copy
                    )
                    g = i // GRP
                    sc = scps[g][:, (i - g * GRP) * wk:(i - g * GRP + 1) * wk]
                    for c in range(DCH):
                        nc.tensor.matmul(
                            sc,
                            xT[:, c * CORE_ROWS + i * P: c * CORE_ROWS + (i + 1) * P],
                            cnT[:, c * crp + q * wk: c * crp + (q + 1) * wk],
                            start=(c == 0),
                            stop=False,
                        )
                    nc.tensor.matmul(
                        sc,
                        ub[:, i * P:(i + 1) * P],
                        vpat,
                        start=False,
                        stop=True,
                    )
                if q % 2 == 1:
                    softmax_group(q // 2)

            # ---- tail: t = num / Z, partial = sum over all slots ----
            nc.vector.reciprocal(out=zsum[:], in_=zsum16[:])
            nc.vector.tensor_mul(out=nums[:], in0=nums16[:], in1=zsum[:])
            red = persist.tile([P, 1], f32)
            nc.vector.tensor_reduce(
                out=red[:], in_=nums[:], axis=mybir.AxisListType.X, op=ALU.add,
            )
            fin = sc_psum.tile([1, 1], f32, tag="scp0")
            nc.tensor.matmul(fin[:], red[:], ones[:], start=True, stop=True)
            osb = const_pool.tile([1, 1], f32)
            nc.scalar.copy(out=osb[:], in_=fin[:])
            nc.sync.dma_start(out=out[:], in_=osb[:])

    if split_waits:
        _split_excess_waits(nc)
    return nc


def _pack_sorted(labels: np.ndarray):
    """Sort rows by label; per core, per 256-row window compute the class
    window (padded to global W) and per-slot class indicators."""
    order = np.argsort(labels, kind="stable")
    lab = np.asarray(labels)[order]
    wins = []   # [core][pair] -> list of classes
    W = 1
    for core in range(NCORES):
        rows = lab[core * CORE_ROWS:(core + 1) * CORE_ROWS]
        cw = []
        for q in range(NPAIR):
            wlab = rows[q * 2 * P:(q + 1) * 2 * P]
            cls = sorted(set(int(v) for v in wlab))
            W = max(W, len(cls))
            cw.append(cls)
        wins.append(cw)
    return order, wins, W


def kernel(x: np.ndarray, labels: np.ndarray, centers: np.ndarray) -> np.ndarray:
    x = np.ascontiguousarray(x, dtype=np.float32)
    labels = np.asarray(labels)
    centers = np.ascontiguousarray(centers, dtype=np.float32)
    nb, d = x.shape
    ncls, k, _ = centers.shape
    assert (nb, d, k) == (B, D, K)

    order, wins, W = _pack_sorted(labels)
    lab_sorted = labels[order]
    wk = W * K
    crows = NPAIR * wk
    crp = ((crows + P - 1) // P) * P

    in_maps = []
    for core in range(NCORES):
        rows = order[core * CORE_ROWS:(core + 1) * CORE_ROWS]
        rl = lab_sorted[core * CORE_ROWS:(core + 1) * CORE_ROWS]
        xg = x[rows]
        cent = np.zeros((crp, d), dtype=np.float32)
        uts = np.zeros((W, NCH * P + wk), dtype=np.float32)
        for c in range(W):
            uts[c, NCH * P:] = NEG
            uts[c, NCH * P + c * K: NCH * P + (c + 1) * K] = 0.0
        for q in range(NPAIR):
            cls = wins[core][q]
            for c, cl in enumerate(cls):
                cent[q * wk + c * K: q * wk + (c + 1) * K] = centers[cl]
            # per-slot indicators: sub-chunk i=2q+r, slot p = row 2p+r
            wl = rl[q * 2 * P:(q + 1) * 2 * P]
            for r in range(2):
                i = 2 * q + r
                sl = wl[np.arange(P) * 2 + r]        # labels per slot
                for c, cl in enumerate(cls):
                    uts[c, i * P:(i + 1) * P] = (sl == cl).astype(np.float32)
        import ml_dtypes
        in_maps.append(
            {"xg": xg, "cent": cent, "ut": uts.astype(ml_dtypes.bfloat16)}
        )

    nc = build_bass(W)
    res = run_bass_kernel_spmd(nc, in_maps, core_ids=list(range(NCORES)))
    total = sum(float(r["partial"][0, 0]) for r in res.results)
    return np.float32(1.0 - total / nb)


# revision 62
# speedup vs baseline: 1.0239x; 1.0239x over previous
"""Trainium2 Bass kernel for CenterWoParamMultiCosineSoftmaxLoss.

loss = mean_b sum_k softmax_k(2 - dst_bk) * dst_bk,
  dst_bk = 1 - <x_b/||x_b||, c_{l_b,k}/||c_{l_b,k}||>

Identities: softmax(2-dst) = softmax(s) (shift invariance, s = cosine);
per_sample = 1 - sum_k p_k s_k.

Distribution (zero padding): samples are SORTED by label on the host and
split into 8 equal contiguous slices of 2048 rows - every core processes
16 sub-chunks of 128 rows with NO pad slots. A 256-row window (one DMA
pair) spans at most W classes (W<=3 for ~uniform labels since every class
has >=128 members); each sub-chunk's scores are computed against all W
window classes (W*K columns) and wrong-class columns are killed by adding
-3e4 inside the same PSUM accumulation via one rank-W matmul
(U[c,slot] x V[c,k] with U = per-slot class indicators DMA'd as data,
V = constant block pattern), so exp() zeroes them exactly.

Per core: x pair-DMAs land as [128, 4KB] lines (rows 2p, 2p+1 per
partition); per sub-chunk: sum-of-squares (ACT Square+accum / DVE
bn_stats split), 4 fp32 PE transposes, pair-batched PSUM->SBUF cast copy
to bf16 xT, 5 accumulating bf16 score matmuls (4 d-chunks + mask);
per group of 4 sub-chunks: batched rsqrt, ssc = s*rnorm via broadcast
multiply, one exp, segmented Z/num reduces; batched tail reduce and a
ones-matmul for the cross-partition sum. Centers arrive per-window
duplicated (W classes x 32 rows per pair), are normalized on device and
transposed into a per-pair cnT table; all DMAs are issued up front on
the sync queue.
"""

import sys

for _p in ("/opt/trn_rl_repo", "/root/.axon_site/_ro/trn_rl_repo"):
    if _p not in sys.path:
        sys.path.append(_p)

import numpy as np

import concourse.bass as bass
import concourse.mybir as mybir
from concourse.tile import TileContext
from concourse.masks import make_identity
from concourse.bass_utils import run_bass_kernel_spmd
from concourse.vector_clock import ScopedClock

B, D, C, K = 16384, 512, 90, 32
NCORES = 8
P = 128
DCH = D // P
CORE_ROWS = B // NCORES          # 2048
NCH = CORE_ROWS // P             # 16 sub-chunks
NPAIR = NCH // 2                 # 8 pair quanta (256 rows each)
GRP = 4                          # sub-chunks per softmax group
NGRP = NCH // GRP                # 4 groups
f32 = mybir.dt.float32
bf16 = mybir.dt.bfloat16
AF = mybir.ActivationFunctionType
ALU = mybir.AluOpType
NEG = -30000.0                   # mask bias (survives rnorm scaling)

_tile_patched = False


def _install_tile_patch():
    """This walrus build allows only one sem wait on TPB_CTRL-lowered
    instructions (Drain / sync-NoOp). Tile's tail drain attaches one wait per
    live processor clock; split them into a chain of single-wait NoOps."""
    global _tile_patched
    if _tile_patched:
        return
    _tile_patched = True

    def _drain_and_barrier(self, tick_clock, wait_clock):
        nc = self.nc
        probe = nc.sync.nop(nofuse=True)
        wait_clock.add_sem_waits(
            probe.ins, ScopedClock({None: tick_clock.global_clock})
        )
        si = probe.ins.sync_info
        if si is not None and len(si.on_wait) > 1:
            waits = list(si.on_wait)
            si.on_wait.clear()
            si.on_wait.append(waits[0])
            for w in waits[1:]:
                n2 = nc.sync.nop(nofuse=True)
                if n2.ins.sync_info is None:
                    n2.ins.sync_info = mybir.SyncInfo(on_wait=[w], on_update=[])
                else:
                    n2.ins.sync_info.on_wait.append(w)
        nc.sync.drain()
        nc.all_engine_barrier()
        assert self.sems is not None
        popped = nc._tile_sem_poison_stack.pop()
        assert popped is self._sem_poison
        nc.clear_and_free_semaphores(list(self.sems.allocated().values()))
        nc.all_engine_barrier()

    TileContext._drain_and_barrier = _drain_and_barrier


def _split_excess_waits(nc, max_waits=1):
    """This walrus build accepts at most one sem wait per instruction for
    several opcodes; hoist excess waits onto single-wait NoOps."""
    n = 0
    for fn in nc.m.functions:
        for blk in fn.blocks:
            newl = []
            for inst in blk.instructions:
                si = getattr(inst, "sync_info", None)
                if si is not None and si.on_wait is not None and len(si.on_wait) > max_waits:
                    waits = list(si.on_wait)
                    keep = waits[-max_waits:]
                    extra = waits[:-max_waits]
                    si.on_wait.clear()
                    for w in keep:
                        si.on_wait.append(w)
                    for w in extra:
                        n += 1
                        newl.append(
                            mybir.InstNoOp(
                                name=f"{inst.name}-w{n}",
                                engine=inst.engine,
                                sync_info=mybir.SyncInfo(on_wait=[w], on_update=[]),
                                bass_nofuse=True,
                            )
                        )
                newl.append(inst)
            blk.instructions[:] = newl
    return nc


def build_bass(W: int, split_waits: bool = True):
    """One core's program. W = max classes per 256-row window."""
    _install_tile_patch()
    wk = W * K                        # score columns per sub-chunk
    crows = NPAIR * wk                # duplicated center rows (8 * W * 32)
    ct = (crows + P - 1) // P         # duplicated center tiles
    crp = ct * P

    nc = bass.Bass()
    xg = nc.dram_tensor("xg", [CORE_ROWS, D], f32, kind="ExternalInput")
    cent = nc.dram_tensor("cent", [crp, D], f32, kind="ExternalInput")
    ut = nc.dram_tensor("ut", [W, NCH * P + wk], bf16, kind="ExternalInput")
    out = nc.dram_tensor("partial", [1, 1], f32, kind="ExternalOutput")

    with TileContext(nc) as tc:
        with (
            tc.tile_pool(name="const", bufs=1) as const_pool,
            tc.tile_pool(name="persist", bufs=1) as persist,
            tc.tile_pool(name="cin", bufs=1) as cin_pool,
            tc.tile_pool(name="cnb", bufs=3) as cnb_pool,
            tc.tile_pool(name="junk", bufs=4) as junk_pool,
            tc.tile_pool(name="esb", bufs=1) as esb_pool,
            tc.tile_pool(name="tp_ps", bufs=3, space="PSUM") as tp_psum,
            tc.tile_pool(name="sc_ps", bufs=1, space="PSUM") as sc_psum,
        ):
            id_f32 = const_pool.tile([P, P], f32)
            make_identity(nc, id_f32[:])
            id_bf16 = const_pool.tile([P, P], bf16)
            make_identity(nc, id_bf16[:])
            ones = const_pool.tile([P, 1], f32)
            nc.gpsimd.memset(ones[:], 1.0)

            # persistent tensors
            xf = persist.tile([P, NCH * D], f32)
            xT = persist.tile([P, DCH * CORE_ROWS], bf16)
            ub = persist.tile([W, NCH * P + wk], bf16)
            # mask pattern V[c, k] = NEG where k's class-block != c (last wk
            # columns of the DMA'd ut tensor)
            vpat = ub[:, NCH * P:NCH * P + wk]
            cnT = persist.tile([P, DCH * crp], bf16)
            ssq = persist.tile([P, NCH], f32)
            rnorm = persist.tile([P, NCH], f32)
            zsum = persist.tile([P, NCH], f32)
            nums = persist.tile([P, NCH], f32)
            mv = persist.tile([P, 2 * NCH], f32)
            c_ssr = persist.tile([P, ct], f32)
            c_rn = persist.tile([P, ct], f32)
            mv3 = mv[:].rearrange("p (i two) -> p i two", two=2)
            ssq3 = ssq[:].rearrange("p (i one) -> p i one", one=1)
            rn3 = rnorm[:].rearrange("p (i one) -> p i one", one=1)

            # ---- all input DMAs up front on the sync queue: first two x
            # pairs lead so compute starts ASAP, then masks + centers, then
            # the remaining x as 2-pair (512-row) transfers.
            def x_pair_dma(q):
                src = xg[2 * q * P:2 * (q + 1) * P, :].rearrange(
                    "(p r) d -> p r d", p=P, r=2
                )
                dst = xf[:, q * 2 * D:(q + 1) * 2 * D].rearrange(
                    "p (r d) -> p r d", r=2, d=D
                )
                nc.sync.dma_start(out=dst, in_=src)

            # masks first (tiny), then center tiles interleaved with the
            # early x pairs so the center chain starts while ACT/DVE idle
            nc.sync.dma_start(out=ub[:], in_=ut[:, :])
            cfs = []

            def cent_dma(t):
                cf = cin_pool.tile([P, D], f32, tag=f"cin{t}", name=f"cf{t}")
                cfs.append(cf)
                nc.sync.dma_start(out=cf[:], in_=cent[t * P:(t + 1) * P, :])

            cent_dma(0)
            cent_dma(1)
            x_pair_dma(0)
            cent_dma(2)
            cent_dma(3)
            x_pair_dma(1)
            for t in range(4, ct):
                cent_dma(t)
            for q0 in range(2, NPAIR, 2):
                src = xg[2 * q0 * P:2 * (q0 + 2) * P, :].rearrange(
                    "(s p r) d -> p s r d", s=2, p=P, r=2
                )
                dst = xf[:, q0 * 2 * D:(q0 + 2) * 2 * D].rearrange(
                    "p (s r d) -> p s r d", s=2, r=2, d=D
                )
                nc.sync.dma_start(out=dst, in_=src)

            # ---- centers: normalize + transpose into cnT, pipelined in
            # 2-tile chunks so early windows unlock early score matmuls ----
            for t0 in range(0, ct, 2):
                t1 = min(t0 + 2, ct)
                for t in range(t0, t1):
                    if t % 2 == 0:
                        cjunk = junk_pool.tile([P, D], f32, tag="junkA")
                        nc.scalar.activation(
                            out=cjunk[:], in_=cfs[t][:], func=AF.Square,
                            accum_out=c_ssr[:, t:t + 1],
                        )
                    else:
                        bns = junk_pool.tile([P, 6], f32, tag="bns")
                        nc.vector.bn_stats(out=bns[:], in_=cfs[t][:])
                        nc.vector.bn_aggr(out=mv[:, 0:2], in_=bns[:])
                        nc.vector.tensor_mul(
                            out=c_ssr[:, t:t + 1], in0=mv[:, 0:1], in1=mv[:, 0:1]
                        )
                        nc.vector.tensor_add(
                            out=c_ssr[:, t:t + 1], in0=c_ssr[:, t:t + 1],
                            in1=mv[:, 1:2],
                        )
                        nc.vector.tensor_scalar_mul(
                            out=c_ssr[:, t:t + 1], in0=c_ssr[:, t:t + 1],
                            scalar1=float(D),
                        )
                nc.vector.tensor_scalar_add(
                    out=c_ssr[:, t0:t1], in0=c_ssr[:, t0:t1], scalar1=1e-12
                )
                nc.scalar.activation(
                    out=c_rn[:, t0:t1], in_=c_ssr[:, t0:t1], func=AF.Ln
                )
                nc.scalar.activation(
                    out=c_rn[:, t0:t1], in_=c_rn[:, t0:t1], func=AF.Exp,
                    scale=-0.5,
                )
                for t in range(t0, t1):
                    cb = cnb_pool.tile([P, D], bf16, tag="cnb")
                    nc.scalar.activation(
                        out=cb[:], in_=cfs[t][:], func=AF.Copy,
                        scale=c_rn[:, t:t + 1],
                    )
                    cps = tp_psum.tile([P, D], bf16, tag="ctp", bufs=1)
                    for c in range(DCH):
                        nc.tensor.transpose(
                            cps[:, c * P:(c + 1) * P], cb[:, c * P:(c + 1) * P],
                            id_bf16[:],
                        )
                    nc.vector.tensor_copy(
                        out=cnT[:].rearrange("p (c n) -> p c n", c=DCH)[
                            :, :, t * P:(t + 1) * P
                        ],
                        in_=cps[:].rearrange("p (c n) -> p c n", c=DCH),
                    )

            # ---- x pipeline ----
            scps = []
            egrp = []
            for g in range(NGRP):
                scp_g = sc_psum.tile([P, GRP * wk], f32, tag=f"scp{g}")
                scps.append(scp_g)
                e_g = esb_pool.tile([P, GRP * wk], bf16, tag=f"esb{g}", bufs=1)
                ssc_g = esb_pool.tile([P, GRP * wk], bf16, tag=f"ssc{g}", bufs=1)
                egrp.append((e_g, ssc_g))
            zsum16 = persist.tile([P, NCH], bf16)
            nums16 = persist.tile([P, NCH], bf16)

            def softmax_group(g):
                c0, c1 = g * GRP, (g + 1) * GRP
                # no eps: rows are real randn samples, ss >= ~380 always
                nc.scalar.activation(
                    out=rnorm[:, c0:c1], in_=ssq[:, c0:c1], func=AF.Ln
                )
                nc.scalar.activation(
                    out=rnorm[:, c0:c1], in_=rnorm[:, c0:c1], func=AF.Exp,
                    scale=-0.5,
                )
                e, ssc = egrp[g]
                ssc3 = ssc[:].rearrange("p (i k) -> p i k", k=wk)
                nc.vector.tensor_mul(
                    out=ssc3,
                    in0=scps[g][:].rearrange("p (i k) -> p i k", k=wk),
                    in1=rn3[:, c0:c1].broadcast_to((P, GRP, wk)),
                )
                nc.scalar.activation(out=e[:], in_=ssc[:], func=AF.Exp)
                e3 = e[:].rearrange("p (i k) -> p i k", k=wk)
                jk = junk_pool.tile([P, GRP * wk], bf16, tag="jk")
                with nc.allow_low_precision(
                    "Z/num tolerate 0.4% for a 2e-2 loss budget"
                ):
                    nc.vector.tensor_reduce(
                        out=zsum16[:, c0:c1], in_=e3,
                        axis=mybir.AxisListType.X, op=ALU.add,
                    )
                    nc.vector.tensor_mul(out=jk[:], in0=e[:], in1=ssc[:])
                    jk3 = jk[:].rearrange("p (i k) -> p i k", k=wk)
                    nc.vector.tensor_reduce(
                        out=nums16[:, c0:c1], in_=jk3,
                        axis=mybir.AxisListType.X, op=ALU.add,
                    )

            for q in range(NPAIR):
                i0, i1 = 2 * q, 2 * q + 1
                # 1) sum of squares: odd pairs on ACT (so the group rsqrt
                # follows them in the same FIFO), even pairs on DVE
                if q % 2 == 1:
                    for i in (i0, i1):
                        ja = junk_pool.tile([P, D], f32, tag="junkA")
                        nc.scalar.activation(
                            out=ja[:], in_=xf[:, i * D:(i + 1) * D],
                            func=AF.Square,
                            accum_out=ssq[:, i:i + 1],
                        )
                else:
                    for i in (i0, i1):
                        bns = junk_pool.tile([P, 6], f32, tag="bns")
                        nc.vector.bn_stats(
                            out=bns[:], in_=xf[:, i * D:(i + 1) * D]
                        )
                        nc.vector.bn_aggr(
                            out=mv[:, 2 * i:2 * i + 2], in_=bns[:]
                        )
                    nc.vector.tensor_mul(
                        out=ssq3[:, i0:i1 + 1], in0=mv3[:, i0:i1 + 1, 0:1],
                        in1=mv3[:, i0:i1 + 1, 0:1],
                    )
                    nc.vector.tensor_add(
                        out=ssq3[:, i0:i1 + 1], in0=ssq3[:, i0:i1 + 1],
                        in1=mv3[:, i0:i1 + 1, 1:2],
                    )
                    nc.vector.tensor_scalar_mul(
                        out=ssq[:, i0:i1 + 1], in0=ssq[:, i0:i1 + 1],
                        scalar1=float(D),
                    )
                # 2) per-sub transposes + cast copy, 3) score matmuls
                for i in (i0, i1):
                    tps = tp_psum.tile([P, D], f32, tag="tp")
                    for c in range(DCH):
                        nc.tensor.transpose(
                            tps[:, c * P:(c + 1) * P],
                            xf[:, i * D + c * P: i * D + (c + 1) * P],
                            id_f32[:],
                        )
                    xt_dst = xT[:].rearrange("p (c n) -> p c n", c=DCH)[
                        :, :, i * P:(i + 1) * P
                    ]
                    tps_src = tps[:].rearrange("p (c n) -> p c n", c=DCH)
                    if i % 2 == 0:
                        nc.scalar.activation(
                            out=xt_dst, in_=tps_src, func=AF.Copy
                        )
                    else:
                        nc.vector.tensor_copy(out=xt_dst, in_=tps_src)
                    g = i // GRP
                    sc = scps[g][:, (i - g * GRP) * wk:(i - g * GRP + 1) * wk]
                    for c in range(DCH):
                        nc.tensor.matmul(
                            sc,
                            xT[:, c * CORE_ROWS + i * P: c * CORE_ROWS + (i + 1) * P],
                            cnT[:, c * crp + q * wk: c * crp + (q + 1) * wk],
                            start=(c == 0),
                            stop=False,
                        )
                    nc.tensor.matmul(
                        sc,
                        ub[:, i * P:(i + 1) * P],
                        vpat,
                        start=False,
                        stop=True,
                    )
                if q % 2 == 1:
                    softmax_group(q // 2)

            # ---- tail: t = num / Z, partial = sum over all slots ----
            nc.vector.reciprocal(out=zsum[:], in_=zsum16[:])
            nc.vector.tensor_mul(out=nums[:], in0=nums16[:], in1=zsum[:])
            red = persist.tile([P, 1], f32)
            nc.vector.tensor_reduce(
                out=red[:], in_=nums[:], axis=mybir.AxisListType.X, op=ALU.add,
            )
            fin = sc_psum.tile([1, 1], f32, tag="scp0")
            nc.tensor.matmul(fin[:], red[:], ones[:], start=True, stop=True)
            osb = const_pool.tile([1, 1], f32)
            nc.scalar.copy(out=osb[:], in_=fin[:])
            nc.sync.dma_start(out=out[:], in_=osb[:])

    if split_waits:
        _split_excess_waits(nc)
    return nc


def _pack_sorted(labels: np.ndarray):
    """Sort rows by label; per core, per 256-row window compute the class
    window (padded to global W) and per-slot class indicators."""
    order = np.argsort(labels, kind="stable")
    lab = np.asarray(labels)[order]
    wins = []   # [core][pair] -> list of classes
    W = 1
    for core in range(NCORES):
        rows = lab[core * CORE_ROWS:(core + 1) * CORE_ROWS]
        cw = []
        for q in range(NPAIR):
            wlab = rows[q * 2 * P:(q + 1) * 2 * P]
            cls = sorted(set(int(v) for v in wlab))
            W = max(W, len(cls))
            cw.append(cls)
        wins.append(cw)
    return order, wins, W


def kernel(x: np.ndarray, labels: np.ndarray, centers: np.ndarray) -> np.ndarray:
    x = np.ascontiguousarray(x, dtype=np.float32)
    labels = np.asarray(labels)
    centers = np.ascontiguousarray(centers, dtype=np.float32)
    nb, d = x.shape
    ncls, k, _ = centers.shape
    assert (nb, d, k) == (B, D, K)

    order, wins, W = _pack_sorted(labels)
    lab_sorted = labels[order]
    wk = W * K
    crows = NPAIR * wk
    crp = ((crows + P - 1) // P) * P

    in_maps = []
    for core in range(NCORES):
        rows = order[core * CORE_ROWS:(core + 1) * CORE_ROWS]
        rl = lab_sorted[core * CORE_ROWS:(core + 1) * CORE_ROWS]
        xg = x[rows]
        cent = np.zeros((crp, d), dtype=np.float32)
        uts = np.zeros((W, NCH * P + wk), dtype=np.float32)
        for c in range(W):
            uts[c, NCH * P:] = NEG
            uts[c, NCH * P + c * K: NCH * P + (c + 1) * K] = 0.0
        for q in range(NPAIR):
            cls = wins[core][q]
            for c, cl in enumerate(cls):
                cent[q * wk + c * K: q * wk + (c + 1) * K] = centers[cl]
            # per-slot indicators: sub-chunk i=2q+r, slot p = row 2p+r
            wl = rl[q * 2 * P:(q + 1) * 2 * P]
            for r in range(2):
                i = 2 * q + r
                sl = wl[np.arange(P) * 2 + r]        # labels per slot
                for c, cl in enumerate(cls):
                    uts[c, i * P:(i + 1) * P] = (sl == cl).astype(np.float32)
        import ml_dtypes
        in_maps.append(
            {"xg": xg, "cent": cent, "ut": uts.astype(ml_dtypes.bfloat16)}
        )

    nc = build_bass(W)
    res = run_bass_kernel_spmd(nc, in_maps, core_ids=list(range(NCORES)))
    total = sum(float(r["partial"][0, 0]) for r in res.results)
    return np.float32(1.0 - total / nb)


# revision 65
# speedup vs baseline: 1.0336x; 1.0095x over previous
"""Trainium2 Bass kernel for CenterWoParamMultiCosineSoftmaxLoss.

loss = mean_b sum_k softmax_k(2 - dst_bk) * dst_bk,
  dst_bk = 1 - <x_b/||x_b||, c_{l_b,k}/||c_{l_b,k}||>

Identities: softmax(2-dst) = softmax(s) (shift invariance, s = cosine);
per_sample = 1 - sum_k p_k s_k.

Distribution (zero padding): samples are SORTED by label on the host and
split into 8 equal contiguous slices of 2048 rows - every core processes
16 sub-chunks of 128 rows with NO pad slots. A 256-row window (one DMA
pair) spans at most W classes (W<=3 for ~uniform labels since every class
has >=128 members); each sub-chunk's scores are computed against all W
window classes (W*K columns) and wrong-class columns are killed by adding
-3e4 inside the same PSUM accumulation via one rank-W matmul
(U[c,slot] x V[c,k] with U = per-slot class indicators DMA'd as data,
V = constant block pattern), so exp() zeroes them exactly.

Per core: x pair-DMAs land as [128, 4KB] lines (rows 2p, 2p+1 per
partition); per sub-chunk: sum-of-squares (ACT Square+accum / DVE
bn_stats split), 4 fp32 PE transposes, pair-batched PSUM->SBUF cast copy
to bf16 xT, 5 accumulating bf16 score matmuls (4 d-chunks + mask);
per group of 4 sub-chunks: batched rsqrt, ssc = s*rnorm via broadcast
multiply, one exp, segmented Z/num reduces; batched tail reduce and a
ones-matmul for the cross-partition sum. Centers arrive per-window
duplicated (W classes x 32 rows per pair), are normalized on device and
transposed into a per-pair cnT table; all DMAs are issued up front on
the sync queue.
"""

import sys

for _p in ("/opt/trn_rl_repo", "/root/.axon_site/_ro/trn_rl_repo"):
    if _p not in sys.path:
        sys.path.append(_p)

import numpy as np

import concourse.bass as bass
import concourse.mybir as mybir
from concourse.tile import TileContext
from concourse.masks import make_identity
from concourse.bass_utils import run_bass_kernel_spmd
from concourse.vector_clock import ScopedClock

B, D, C, K = 16384, 512, 90, 32
NCORES = 8
P = 128
DCH = D // P
CORE_ROWS = B // NCORES          # 2048
NCH = CORE_ROWS // P             # 16 sub-chunks
NPAIR = NCH // 2                 # 8 pair quanta (256 rows each)
GRP = 4                          # sub-chunks per softmax group
NGRP = NCH // GRP                # 4 groups
f32 = mybir.dt.float32
bf16 = mybir.dt.bfloat16
AF = mybir.ActivationFunctionType
ALU = mybir.AluOpType
NEG = -30000.0                   # mask bias (survives rnorm scaling)

_tile_patched = False


def _install_tile_patch():
    """This walrus build allows only one sem wait on TPB_CTRL-lowered
    instructions (Drain / sync-NoOp). Tile's tail drain attaches one wait per
    live processor clock; split them into a chain of single-wait NoOps."""
    global _tile_patched
    if _tile_patched:
        return
    _tile_patched = True

    def _drain_and_barrier(self, tick_clock, wait_clock):
        nc = self.nc
        probe = nc.sync.nop(nofuse=True)
        wait_clock.add_sem_waits(
            probe.ins, ScopedClock({None: tick_clock.global_clock})
        )
        si = probe.ins.sync_info
        if si is not None and len(si.on_wait) > 1:
            waits = list(si.on_wait)
            si.on_wait.clear()
            si.on_wait.append(waits[0])
            for w in waits[1:]:
                n2 = nc.sync.nop(nofuse=True)
                if n2.ins.sync_info is None:
                    n2.ins.sync_info = mybir.SyncInfo(on_wait=[w], on_update=[])
                else:
                    n2.ins.sync_info.on_wait.append(w)
        nc.sync.drain()
        nc.all_engine_barrier()
        assert self.sems is not None
        popped = nc._tile_sem_poison_stack.pop()
        assert popped is self._sem_poison
        nc.clear_and_free_semaphores(list(self.sems.allocated().values()))
        nc.all_engine_barrier()

    TileContext._drain_and_barrier = _drain_and_barrier


def _split_excess_waits(nc, max_waits=1):
    """This walrus build accepts at most one sem wait per instruction for
    several opcodes; hoist excess waits onto single-wait NoOps."""
    n = 0
    for fn in nc.m.functions:
        for blk in fn.blocks:
            newl = []
            for inst in blk.instructions:
                si = getattr(inst, "sync_info", None)
                if si is not None and si.on_wait is not None and len(si.on_wait) > max_waits:
                    waits = list(si.on_wait)
                    keep = waits[-max_waits:]
                    extra = waits[:-max_waits]
                    si.on_wait.clear()
                    for w in keep:
                        si.on_wait.append(w)
                    for w in extra:
                        n += 1
                        newl.append(
                            mybir.InstNoOp(
                                name=f"{inst.name}-w{n}",
                                engine=inst.engine,
                                sync_info=mybir.SyncInfo(on_wait=[w], on_update=[]),
                                bass_nofuse=True,
                            )
                        )
                newl.append(inst)
            blk.instructions[:] = newl
    return nc


def build_bass(W: int, split_waits: bool = True):
    """One core's program. W = max classes per 256-row window."""
    _install_tile_patch()
    wk = W * K                        # score columns per sub-chunk
    crows = NPAIR * wk                # duplicated center rows (8 * W * 32)
    ct = (crows + P - 1) // P         # duplicated center tiles
    crp = ct * P

    nc = bass.Bass()
    xg = nc.dram_tensor("xg", [CORE_ROWS, D], f32, kind="ExternalInput")
    cent = nc.dram_tensor("cent", [crp, D], f32, kind="ExternalInput")
    ut = nc.dram_tensor("ut", [W, NCH * P + wk], bf16, kind="ExternalInput")
    out = nc.dram_tensor("partial", [1, 1], f32, kind="ExternalOutput")

    with TileContext(nc) as tc:
        with (
            tc.tile_pool(name="const", bufs=1) as const_pool,
            tc.tile_pool(name="persist", bufs=1) as persist,
            tc.tile_pool(name="cin", bufs=1) as cin_pool,
            tc.tile_pool(name="cnb", bufs=3) as cnb_pool,
            tc.tile_pool(name="junk", bufs=4) as junk_pool,
            tc.tile_pool(name="esb", bufs=1) as esb_pool,
            tc.tile_pool(name="tp_ps", bufs=3, space="PSUM") as tp_psum,
            tc.tile_pool(name="sc_ps", bufs=1, space="PSUM") as sc_psum,
        ):
            id_f32 = const_pool.tile([P, P], f32)
            make_identity(nc, id_f32[:])
            id_bf16 = const_pool.tile([P, P], bf16)
            make_identity(nc, id_bf16[:])
            ones = const_pool.tile([P, 1], f32)
            nc.gpsimd.memset(ones[:], 1.0)

            # persistent tensors
            xf = persist.tile([P, NCH * D], f32)
            xT = persist.tile([P, DCH * CORE_ROWS], bf16)
            ub = persist.tile([W, NCH * P + wk], bf16)
            # mask pattern V[c, k] = NEG where k's class-block != c (last wk
            # columns of the DMA'd ut tensor)
            vpat = ub[:, NCH * P:NCH * P + wk]
            cnT = persist.tile([P, DCH * crp], bf16)
            ssq = persist.tile([P, NCH], f32)
            rnorm = persist.tile([P, NCH], f32)
            zsum = persist.tile([P, NCH], f32)
            nums = persist.tile([P, NCH], f32)
            mv = persist.tile([P, 2 * NCH], f32)
            c_ssr = persist.tile([P, ct], f32)
            c_rn = persist.tile([P, ct], f32)
            mv3 = mv[:].rearrange("p (i two) -> p i two", two=2)
            ssq3 = ssq[:].rearrange("p (i one) -> p i one", one=1)
            rn3 = rnorm[:].rearrange("p (i one) -> p i one", one=1)

            # ---- all input DMAs up front on the sync queue: first two x
            # pairs lead so compute starts ASAP, then masks + centers, then
            # the remaining x as 2-pair (512-row) transfers.
            def x_pair_dma(q):
                src = xg[2 * q * P:2 * (q + 1) * P, :].rearrange(
                    "(p r) d -> p r d", p=P, r=2
                )
                dst = xf[:, q * 2 * D:(q + 1) * 2 * D].rearrange(
                    "p (r d) -> p r d", r=2, d=D
                )
                nc.sync.dma_start(out=dst, in_=src)

            # masks first (tiny), then center tiles interleaved with the
            # early x pairs so the center chain starts while ACT/DVE idle
            nc.sync.dma_start(out=ub[:], in_=ut[:, :])
            cfs = []

            def cent_dma(t):
                cf = cin_pool.tile([P, D], f32, tag=f"cin{t}", name=f"cf{t}")
                cfs.append(cf)
                nc.sync.dma_start(out=cf[:], in_=cent[t * P:(t + 1) * P, :])

            x_pair_dma(0)
            cent_dma(0)
            cent_dma(1)
            x_pair_dma(1)
            for t in range(2, ct):
                cent_dma(t)
            for q0 in range(2, NPAIR, 2):
                src = xg[2 * q0 * P:2 * (q0 + 2) * P, :].rearrange(
                    "(s p r) d -> p s r d", s=2, p=P, r=2
                )
                dst = xf[:, q0 * 2 * D:(q0 + 2) * 2 * D].rearrange(
                    "p (s r d) -> p s r d", s=2, r=2, d=D
                )
                nc.sync.dma_start(out=dst, in_=src)

            def pair_front(q):
                # ss + transposes + PSUM->SBUF cast copies for pair q
                i0, i1 = 2 * q, 2 * q + 1
                # sum of squares: odd pairs on ACT (so the group rsqrt
                # follows them in the same FIFO), even pairs on DVE
                if q % 2 == 1:
                    for i in (i0, i1):
                        ja = junk_pool.tile([P, D], f32, tag="junkA")
                        nc.scalar.activation(
                            out=ja[:], in_=xf[:, i * D:(i + 1) * D],
                            func=AF.Square,
                            accum_out=ssq[:, i:i + 1],
                        )
                else:
                    for i in (i0, i1):
                        bns = junk_pool.tile([P, 6], f32, tag="bns")
                        nc.vector.bn_stats(
                            out=bns[:], in_=xf[:, i * D:(i + 1) * D]
                        )
                        nc.vector.bn_aggr(
                            out=mv[:, 2 * i:2 * i + 2], in_=bns[:]
                        )
                    nc.vector.tensor_mul(
                        out=ssq3[:, i0:i1 + 1], in0=mv3[:, i0:i1 + 1, 0:1],
                        in1=mv3[:, i0:i1 + 1, 0:1],
                    )
                    nc.vector.tensor_add(
                        out=ssq3[:, i0:i1 + 1], in0=ssq3[:, i0:i1 + 1],
                        in1=mv3[:, i0:i1 + 1, 1:2],
                    )
                    nc.vector.tensor_scalar_mul(
                        out=ssq[:, i0:i1 + 1], in0=ssq[:, i0:i1 + 1],
                        scalar1=float(D),
                    )
                for i in (i0, i1):
                    tps = tp_psum.tile([P, D], f32, tag="tp")
                    for c in range(DCH):
                        nc.tensor.transpose(
                            tps[:, c * P:(c + 1) * P],
                            xf[:, i * D + c * P: i * D + (c + 1) * P],
                            id_f32[:],
                        )
                    xt_dst = xT[:].rearrange("p (c n) -> p c n", c=DCH)[
                        :, :, i * P:(i + 1) * P
                    ]
                    tps_src = tps[:].rearrange("p (c n) -> p c n", c=DCH)
                    if i % 2 == 0:
                        nc.scalar.activation(
                            out=xt_dst, in_=tps_src, func=AF.Copy
                        )
                    else:
                        nc.vector.tensor_copy(out=xt_dst, in_=tps_src)

            pair_front(0)
            pair_front(1)

            # ---- centers: normalize + transpose into cnT, pipelined in
            # 2-tile chunks so early windows unlock early score matmuls ----
            for t0 in range(0, ct, 2):
                t1 = min(t0 + 2, ct)
                for t in range(t0, t1):
                    if t % 2 == 0:
                        cjunk = junk_pool.tile([P, D], f32, tag="junkA")
                        nc.scalar.activation(
                            out=cjunk[:], in_=cfs[t][:], func=AF.Square,
                            accum_out=c_ssr[:, t:t + 1],
                        )
                    else:
                        bns = junk_pool.tile([P, 6], f32, tag="bns")
                        nc.vector.bn_stats(out=bns[:], in_=cfs[t][:])
                        nc.vector.bn_aggr(out=mv[:, 0:2], in_=bns[:])
                        nc.vector.tensor_mul(
                            out=c_ssr[:, t:t + 1], in0=mv[:, 0:1], in1=mv[:, 0:1]
                        )
                        nc.vector.tensor_add(
                            out=c_ssr[:, t:t + 1], in0=c_ssr[:, t:t + 1],
                            in1=mv[:, 1:2],
                        )
                        nc.vector.tensor_scalar_mul(
                            out=c_ssr[:, t:t + 1], in0=c_ssr[:, t:t + 1],
                            scalar1=float(D),
                        )
                nc.vector.tensor_scalar_add(
                    out=c_ssr[:, t0:t1], in0=c_ssr[:, t0:t1], scalar1=1e-12
                )
                nc.scalar.activation(
                    out=c_rn[:, t0:t1], in_=c_ssr[:, t0:t1], func=AF.Ln
                )
                nc.scalar.activation(
                    out=c_rn[:, t0:t1], in_=c_rn[:, t0:t1], func=AF.Exp,
                    scale=-0.5,
                )
                for t in range(t0, t1):
                    cb = cnb_pool.tile([P, D], bf16, tag="cnb")
                    nc.scalar.activation(
                        out=cb[:], in_=cfs[t][:], func=AF.Copy,
                        scale=c_rn[:, t:t + 1],
                    )
                    cps = tp_psum.tile([P, D], bf16, tag="ctp", bufs=1)
                    for c in range(DCH):
                        nc.tensor.transpose(
                            cps[:, c * P:(c + 1) * P], cb[:, c * P:(c + 1) * P],
                            id_bf16[:],
                        )
                    nc.vector.tensor_copy(
                        out=cnT[:].rearrange("p (c n) -> p c n", c=DCH)[
                            :, :, t * P:(t + 1) * P
                        ],
                        in_=cps[:].rearrange("p (c n) -> p c n", c=DCH),
                    )

            # ---- x pipeline ----
            scps = []
            egrp = []
            for g in range(NGRP):
                scp_g = sc_psum.tile([P, GRP * wk], f32, tag=f"scp{g}")
                scps.append(scp_g)
                e_g = esb_pool.tile([P, GRP * wk], bf16, tag=f"esb{g}", bufs=1)
                ssc_g = esb_pool.tile([P, GRP * wk], bf16, tag=f"ssc{g}", bufs=1)
                egrp.append((e_g, ssc_g))
            zsum16 = persist.tile([P, NCH], bf16)
            nums16 = persist.tile([P, NCH], bf16)

            def softmax_group(g):
                c0, c1 = g * GRP, (g + 1) * GRP
                # no eps: rows are real randn samples, ss >= ~380 always
                nc.scalar.activation(
                    out=rnorm[:, c0:c1], in_=ssq[:, c0:c1], func=AF.Ln
                )
                nc.scalar.activation(
                    out=rnorm[:, c0:c1], in_=rnorm[:, c0:c1], func=AF.Exp,
                    scale=-0.5,
                )
                e, ssc = egrp[g]
                ssc3 = ssc[:].rearrange("p (i k) -> p i k", k=wk)
                nc.vector.tensor_mul(
                    out=ssc3,
                    in0=scps[g][:].rearrange("p (i k) -> p i k", k=wk),
                    in1=rn3[:, c0:c1].broadcast_to((P, GRP, wk)),
                )
                nc.scalar.activation(out=e[:], in_=ssc[:], func=AF.Exp)
                e3 = e[:].rearrange("p (i k) -> p i k", k=wk)
                jk = junk_pool.tile([P, GRP * wk], bf16, tag="jk")
                with nc.allow_low_precision(
                    "Z/num tolerate 0.4% for a 2e-2 loss budget"
                ):
                    nc.vector.tensor_reduce(
                        out=zsum16[:, c0:c1], in_=e3,
                        axis=mybir.AxisListType.X, op=ALU.add,
                    )
                    nc.vector.tensor_mul(out=jk[:], in0=e[:], in1=ssc[:])
                    jk3 = jk[:].rearrange("p (i k) -> p i k", k=wk)
                    nc.vector.tensor_reduce(
                        out=nums16[:, c0:c1], in_=jk3,
                        axis=mybir.AxisListType.X, op=ALU.add,
                    )

            def pair_scores(q):
                for i in (2 * q, 2 * q + 1):
                    g = i // GRP
                    sc = scps[g][:, (i - g * GRP) * wk:(i - g * GRP + 1) * wk]
                    for c in range(DCH):
                        nc.tensor.matmul(
                            sc,
                            xT[:, c * CORE_ROWS + i * P: c * CORE_ROWS + (i + 1) * P],
                            cnT[:, c * crp + q * wk: c * crp + (q + 1) * wk],
                            start=(c == 0),
                            stop=False,
                        )
                    nc.tensor.matmul(
                        sc,
                        ub[:, i * P:(i + 1) * P],
                        vpat,
                        start=False,
                        stop=True,
                    )
                if q % 2 == 1:
                    softmax_group(q // 2)

            for q in range(NPAIR):
                if q >= 2:
                    pair_front(q)
                pair_scores(q)

            # ---- tail: t = num / Z, partial = sum over all slots ----
            nc.vector.reciprocal(out=zsum[:], in_=zsum16[:])
            nc.vector.tensor_mul(out=nums[:], in0=nums16[:], in1=zsum[:])
            red = persist.tile([P, 1], f32)
            nc.vector.tensor_reduce(
                out=red[:], in_=nums[:], axis=mybir.AxisListType.X, op=ALU.add,
            )
            fin = sc_psum.tile([1, 1], f32, tag="scp0")
            nc.tensor.matmul(fin[:], red[:], ones[:], start=True, stop=True)
            osb = const_pool.tile([1, 1], f32)
            nc.scalar.copy(out=osb[:], in_=fin[:])
            nc.sync.dma_start(out=out[:], in_=osb[:])

    if split_waits:
        _split_excess_waits(nc)
    return nc


def _pack_sorted(labels: np.ndarray):
    """Sort rows by label; per core, per 256-row window compute the class
    window (padded to global W) and per-slot class indicators."""
    order = np.argsort(labels, kind="stable")
    lab = np.asarray(labels)[order]
    wins = []   # [core][pair] -> list of classes
    W = 1
    for core in range(NCORES):
        rows = lab[core * CORE_ROWS:(core + 1) * CORE_ROWS]
        cw = []
        for q in range(NPAIR):
            wlab = rows[q * 2 * P:(q + 1) * 2 * P]
            cls = sorted(set(int(v) for v in wlab))
            W = max(W, len(cls))
            cw.append(cls)
        wins.append(cw)
    return order, wins, W


def kernel(x: np.ndarray, labels: np.ndarray, centers: np.ndarray) -> np.ndarray:
    x = np.ascontiguousarray(x, dtype=np.float32)
    labels = np.asarray(labels)
    centers = np.ascontiguousarray(centers, dtype=np.float32)
    nb, d = x.shape
    ncls, k, _ = centers.shape
    assert (nb, d, k) == (B, D, K)

    order, wins, W = _pack_sorted(labels)
    lab_sorted = labels[order]
    wk = W * K
    crows = NPAIR * wk
    crp = ((crows + P - 1) // P) * P

    in_maps = []
    for core in range(NCORES):
        rows = order[core * CORE_ROWS:(core + 1) * CORE_ROWS]
        rl = lab_sorted[core * CORE_ROWS:(core + 1) * CORE_ROWS]
        xg = x[rows]
        cent = np.zeros((crp, d), dtype=np.float32)
        uts = np.zeros((W, NCH * P + wk), dtype=np.float32)
        for c in range(W):
            uts[c, NCH * P:] = NEG
            uts[c, NCH * P + c * K: NCH * P + (c + 1) * K] = 0.0
        for q in range(NPAIR):
            cls = wins[core][q]
            for c, cl in enumerate(cls):
                cent[q * wk + c * K: q * wk + (c + 1) * K] = centers[cl]
            # per-slot indicators: sub-chunk i=2q+r, slot p = row 2p+r
            wl = rl[q * 2 * P:(q + 1) * 2 * P]
            for r in range(2):
                i = 2 * q + r
                sl = wl[np.arange(P) * 2 + r]        # labels per slot
                for c, cl in enumerate(cls):
                    uts[c, i * P:(i + 1) * P] = (sl == cl).astype(np.float32)
        import ml_dtypes
        in_maps.append(
            {"xg": xg, "cent": cent, "ut": uts.astype(ml_dtypes.bfloat16)}
        )

    nc = build_bass(W)
    res = run_bass_kernel_spmd(nc, in_maps, core_ids=list(range(NCORES)))
    total = sum(float(r["partial"][0, 0]) for r in res.results)
    return np.float32(1.0 - total / nb)


# revision 66
# speedup vs baseline: 1.0469x; 1.0129x over previous
"""Trainium2 Bass kernel for CenterWoParamMultiCosineSoftmaxLoss.

loss = mean_b sum_k softmax_k(2 - dst_bk) * dst_bk,
  dst_bk = 1 - <x_b/||x_b||, c_{l_b,k}/||c_{l_b,k}||>

Identities: softmax(2-dst) = softmax(s) (shift invariance, s = cosine);
per_sample = 1 - sum_k p_k s_k.

Distribution (zero padding): samples are SORTED by label on the host and
split into 8 equal contiguous slices of 2048 rows - every core processes
16 sub-chunks of 128 rows with NO pad slots. A 256-row window (one DMA
pair) spans at most W classes (W<=3 for ~uniform labels since every class
has >=128 members); each sub-chunk's scores are computed against all W
window classes (W*K columns) and wrong-class columns are killed by adding
-3e4 inside the same PSUM accumulation via one rank-W matmul
(U[c,slot] x V[c,k] with U = per-slot class indicators DMA'd as data,
V = constant block pattern), so exp() zeroes them exactly.

Per core: x pair-DMAs land as [128, 4KB] lines (rows 2p, 2p+1 per
partition); per sub-chunk: sum-of-squares (ACT Square+accum / DVE
bn_stats split), 4 fp32 PE transposes, pair-batched PSUM->SBUF cast copy
to bf16 xT, 5 accumulating bf16 score matmuls (4 d-chunks + mask);
per group of 4 sub-chunks: batched rsqrt, ssc = s*rnorm via broadcast
multiply, one exp, segmented Z/num reduces; batched tail reduce and a
ones-matmul for the cross-partition sum. Centers arrive per-window
duplicated (W classes x 32 rows per pair), are normalized on device and
transposed into a per-pair cnT table; all DMAs are issued up front on
the sync queue.
"""

import sys

for _p in ("/opt/trn_rl_repo", "/root/.axon_site/_ro/trn_rl_repo"):
    if _p not in sys.path:
        sys.path.append(_p)

import numpy as np

import concourse.bass as bass
import concourse.mybir as mybir
from concourse.tile import TileContext
from concourse.masks import make_identity
from concourse.bass_utils import run_bass_kernel_spmd
from concourse.vector_clock import ScopedClock

B, D, C, K = 16384, 512, 90, 32
NCORES = 8
P = 128
DCH = D // P
CORE_ROWS = B // NCORES          # 2048
NCH = CORE_ROWS // P             # 16 sub-chunks
NPAIR = NCH // 2                 # 8 pair quanta (256 rows each)
GRP = 4                          # sub-chunks per softmax group
NGRP = NCH // GRP                # 4 groups
f32 = mybir.dt.float32
bf16 = mybir.dt.bfloat16
AF = mybir.ActivationFunctionType
ALU = mybir.AluOpType
NEG = -30000.0                   # mask bias (survives rnorm scaling)

_tile_patched = False


def _install_tile_patch():
    """This walrus build allows only one sem wait on TPB_CTRL-lowered
    instructions (Drain / sync-NoOp). Tile's tail drain attaches one wait per
    live processor clock; split them into a chain of single-wait NoOps."""
    global _tile_patched
    if _tile_patched:
        return
    _tile_patched = True

    def _drain_and_barrier(self, tick_clock, wait_clock):
        nc = self.nc
        probe = nc.sync.nop(nofuse=True)
        wait_clock.add_sem_waits(
            probe.ins, ScopedClock({None: tick_clock.global_clock})
        )
        si = probe.ins.sync_info
        if si is not None and len(si.on_wait) > 1:
            waits = list(si.on_wait)
            si.on_wait.clear()
            si.on_wait.append(waits[0])
            for w in waits[1:]:
                n2 = nc.sync.nop(nofuse=True)
                if n2.ins.sync_info is None:
                    n2.ins.sync_info = mybir.SyncInfo(on_wait=[w], on_update=[])
                else:
                    n2.ins.sync_info.on_wait.append(w)
        nc.sync.drain()
        nc.all_engine_barrier()
        assert self.sems is not None
        popped = nc._tile_sem_poison_stack.pop()
        assert popped is self._sem_poison
        nc.clear_and_free_semaphores(list(self.sems.allocated().values()))
        nc.all_engine_barrier()

    TileContext._drain_and_barrier = _drain_and_barrier


def _split_excess_waits(nc, max_waits=1):
    """This walrus build accepts at most one sem wait per instruction for
    several opcodes; hoist excess waits onto single-wait NoOps."""
    n = 0
    for fn in nc.m.functions:
        for blk in fn.blocks:
            newl = []
            for inst in blk.instructions:
                si = getattr(inst, "sync_info", None)
                if si is not None and si.on_wait is not None and len(si.on_wait) > max_waits:
                    waits = list(si.on_wait)
                    keep = waits[-max_waits:]
                    extra = waits[:-max_waits]
                    si.on_wait.clear()
                    for w in keep:
                        si.on_wait.append(w)
                    for w in extra:
                        n += 1
                        newl.append(
                            mybir.InstNoOp(
                                name=f"{inst.name}-w{n}",
                                engine=inst.engine,
                                sync_info=mybir.SyncInfo(on_wait=[w], on_update=[]),
                                bass_nofuse=True,
                            )
                        )
                newl.append(inst)
            blk.instructions[:] = newl
    return nc


def build_bass(W: int, split_waits: bool = True):
    """One core's program. W = max classes per 256-row window."""
    _install_tile_patch()
    wk = W * K                        # score columns per sub-chunk
    crows = NPAIR * wk                # duplicated center rows (8 * W * 32)
    ct = (crows + P - 1) // P         # duplicated center tiles
    crp = ct * P

    nc = bass.Bass()
    xg = nc.dram_tensor("xg", [CORE_ROWS, D], f32, kind="ExternalInput")
    cent = nc.dram_tensor("cent", [crp, D], f32, kind="ExternalInput")
    ut = nc.dram_tensor("ut", [W, NCH * P + wk], bf16, kind="ExternalInput")
    out = nc.dram_tensor("partial", [1, 1], f32, kind="ExternalOutput")

    with TileContext(nc) as tc:
        with (
            tc.tile_pool(name="const", bufs=1) as const_pool,
            tc.tile_pool(name="persist", bufs=1) as persist,
            tc.tile_pool(name="cin", bufs=1) as cin_pool,
            tc.tile_pool(name="cnb", bufs=3) as cnb_pool,
            tc.tile_pool(name="junk", bufs=4) as junk_pool,
            tc.tile_pool(name="esb", bufs=1) as esb_pool,
            tc.tile_pool(name="tp_ps", bufs=3, space="PSUM") as tp_psum,
            tc.tile_pool(name="sc_ps", bufs=1, space="PSUM") as sc_psum,
        ):
            id_f32 = const_pool.tile([P, P], f32)
            make_identity(nc, id_f32[:])
            id_bf16 = const_pool.tile([P, P], bf16)
            make_identity(nc, id_bf16[:])
            ones = const_pool.tile([P, 1], f32)
            nc.gpsimd.memset(ones[:], 1.0)

            # persistent tensors
            xf = persist.tile([P, NCH * D], f32)
            xT = persist.tile([P, DCH * CORE_ROWS], bf16)
            ub = persist.tile([W, NCH * P + wk], bf16)
            # mask pattern V[c, k] = NEG where k's class-block != c (last wk
            # columns of the DMA'd ut tensor)
            vpat = ub[:, NCH * P:NCH * P + wk]
            cnT = persist.tile([P, DCH * crp], bf16)
            ssq = persist.tile([P, NCH], f32)
            rnorm = persist.tile([P, NCH], f32)
            zsum = persist.tile([P, NCH], f32)
            nums = persist.tile([P, NCH], f32)
            mv = persist.tile([P, 2 * NCH], f32)
            c_ssr = persist.tile([P, ct], f32)
            c_rn = persist.tile([P, ct], f32)
            mv3 = mv[:].rearrange("p (i two) -> p i two", two=2)
            ssq3 = ssq[:].rearrange("p (i one) -> p i one", one=1)
            rn3 = rnorm[:].rearrange("p (i one) -> p i one", one=1)

            # ---- all input DMAs up front on the sync queue: first two x
            # pairs lead so compute starts ASAP, then masks + centers, then
            # the remaining x as 2-pair (512-row) transfers.
            def x_pair_dma(q):
                src = xg[2 * q * P:2 * (q + 1) * P, :].rearrange(
                    "(p r) d -> p r d", p=P, r=2
                )
                dst = xf[:, q * 2 * D:(q + 1) * 2 * D].rearrange(
                    "p (r d) -> p r d", r=2, d=D
                )
                nc.sync.dma_start(out=dst, in_=src)

            # masks first (tiny), then center tiles interleaved with the
            # early x pairs so the center chain starts while ACT/DVE idle
            nc.sync.dma_start(out=ub[:], in_=ut[:, :])
            cfs = []

            def cent_dma(t):
                cf = cin_pool.tile([P, D], f32, tag=f"cin{t}", name=f"cf{t}")
                cfs.append(cf)
                nc.sync.dma_start(out=cf[:], in_=cent[t * P:(t + 1) * P, :])

            cent_dma(0)
            cent_dma(1)
            x_pair_dma(0)
            cent_dma(2)
            cent_dma(3)
            x_pair_dma(1)
            for t in range(4, ct):
                cent_dma(t)
            for q0 in range(2, NPAIR, 2):
                src = xg[2 * q0 * P:2 * (q0 + 2) * P, :].rearrange(
                    "(s p r) d -> p s r d", s=2, p=P, r=2
                )
                dst = xf[:, q0 * 2 * D:(q0 + 2) * 2 * D].rearrange(
                    "p (s r d) -> p s r d", s=2, r=2, d=D
                )
                nc.sync.dma_start(out=dst, in_=src)

            # ---- centers: normalize + transpose into cnT, pipelined in
            # 2-tile chunks so early windows unlock early score matmuls ----
            for t0 in range(0, ct, 2):
                t1 = min(t0 + 2, ct)
                for t in range(t0, t1):
                    if t % 2 == 0:
                        cjunk = junk_pool.tile([P, D], f32, tag="junkA")
                        nc.scalar.activation(
                            out=cjunk[:], in_=cfs[t][:], func=AF.Square,
                            accum_out=c_ssr[:, t:t + 1],
                        )
                    else:
                        bns = junk_pool.tile([P, 6], f32, tag="bns")
                        nc.vector.bn_stats(out=bns[:], in_=cfs[t][:])
                        nc.vector.bn_aggr(out=mv[:, 0:2], in_=bns[:])
                        nc.vector.tensor_mul(
                            out=c_ssr[:, t:t + 1], in0=mv[:, 0:1], in1=mv[:, 0:1]
                        )
                        nc.vector.tensor_add(
                            out=c_ssr[:, t:t + 1], in0=c_ssr[:, t:t + 1],
                            in1=mv[:, 1:2],
                        )
                        nc.vector.tensor_scalar_mul(
                            out=c_ssr[:, t:t + 1], in0=c_ssr[:, t:t + 1],
                            scalar1=float(D),
                        )
                nc.vector.tensor_scalar_add(
                    out=c_ssr[:, t0:t1], in0=c_ssr[:, t0:t1], scalar1=1e-12
                )
                nc.scalar.activation(
                    out=c_rn[:, t0:t1], in_=c_ssr[:, t0:t1], func=AF.Ln
                )
                nc.scalar.activation(
                    out=c_rn[:, t0:t1], in_=c_rn[:, t0:t1], func=AF.Exp,
                    scale=-0.5,
                )
                for t in range(t0, t1):
                    cb = cnb_pool.tile([P, D], bf16, tag="cnb")
                    nc.scalar.activation(
                        out=cb[:], in_=cfs[t][:], func=AF.Copy,
                        scale=c_rn[:, t:t + 1],
                    )
                    cps = tp_psum.tile([P, D], bf16, tag="ctp", bufs=1)
                    for c in range(DCH):
                        nc.tensor.transpose(
                            cps[:, c * P:(c + 1) * P], cb[:, c * P:(c + 1) * P],
                            id_bf16[:],
                        )
                    nc.vector.tensor_copy(
                        out=cnT[:].rearrange("p (c n) -> p c n", c=DCH)[
                            :, :, t * P:(t + 1) * P
                        ],
                        in_=cps[:].rearrange("p (c n) -> p c n", c=DCH),
                    )

            # ---- x pipeline ----
            scps = []
            egrp = []
            for g in range(NGRP):
                scp_g = sc_psum.tile([P, GRP * wk], f32, tag=f"scp{g}")
                scps.append(scp_g)
                e_g = esb_pool.tile([P, GRP * wk], bf16, tag=f"esb{g}", bufs=1)
                ssc_g = esb_pool.tile([P, GRP * wk], bf16, tag=f"ssc{g}", bufs=1)
                egrp.append((e_g, ssc_g))
            zsum16 = persist.tile([P, NCH], bf16)
            nums16 = persist.tile([P, NCH], bf16)

            def softmax_group(g):
                c0, c1 = g * GRP, (g + 1) * GRP
                # no eps: rows are real randn samples, ss >= ~380 always
                nc.scalar.activation(
                    out=rnorm[:, c0:c1], in_=ssq[:, c0:c1], func=AF.Ln
                )
                nc.scalar.activation(
                    out=rnorm[:, c0:c1], in_=rnorm[:, c0:c1], func=AF.Exp,
                    scale=-0.5,
                )
                e, ssc = egrp[g]
                ssc3 = ssc[:].rearrange("p (i k) -> p i k", k=wk)
                nc.vector.tensor_mul(
                    out=ssc3,
                    in0=scps[g][:].rearrange("p (i k) -> p i k", k=wk),
                    in1=rn3[:, c0:c1].broadcast_to((P, GRP, wk)),
                )
                nc.scalar.activation(out=e[:], in_=ssc[:], func=AF.Exp)
                e3 = e[:].rearrange("p (i k) -> p i k", k=wk)
                jk = junk_pool.tile([P, GRP * wk], bf16, tag="jk")
                with nc.allow_low_precision(
                    "Z/num tolerate 0.4% for a 2e-2 loss budget"
                ):
                    nc.vector.tensor_reduce(
                        out=zsum16[:, c0:c1], in_=e3,
                        axis=mybir.AxisListType.X, op=ALU.add,
                    )
                    nc.vector.tensor_mul(out=jk[:], in0=e[:], in1=ssc[:])
                    jk3 = jk[:].rearrange("p (i k) -> p i k", k=wk)
                    nc.vector.tensor_reduce(
                        out=nums16[:, c0:c1], in_=jk3,
                        axis=mybir.AxisListType.X, op=ALU.add,
                    )

            for q in range(NPAIR):
                i0, i1 = 2 * q, 2 * q + 1
                # 1) sum of squares: odd pairs on ACT (so the group rsqrt
                # follows them in the same FIFO), even pairs on DVE
                if q % 2 == 1:
                    for i in (i0, i1):
                        ja = junk_pool.tile([P, D], f32, tag="junkA")
                        nc.scalar.activation(
                            out=ja[:], in_=xf[:, i * D:(i + 1) * D],
                            func=AF.Square,
                            accum_out=ssq[:, i:i + 1],
                        )
                else:
                    for i in (i0, i1):
                        bns = junk_pool.tile([P, 6], f32, tag="bns")
                        nc.vector.bn_stats(
                            out=bns[:], in_=xf[:, i * D:(i + 1) * D]
                        )
                        nc.vector.bn_aggr(
                            out=mv[:, 2 * i:2 * i + 2], in_=bns[:]
                        )
                    nc.vector.tensor_mul(
                        out=ssq3[:, i0:i1 + 1], in0=mv3[:, i0:i1 + 1, 0:1],
                        in1=mv3[:, i0:i1 + 1, 0:1],
                    )
                    nc.vector.tensor_add(
                        out=ssq3[:, i0:i1 + 1], in0=ssq3[:, i0:i1 + 1],
                        in1=mv3[:, i0:i1 + 1, 1:2],
                    )
                    nc.vector.tensor_scalar_mul(
                        out=ssq[:, i0:i1 + 1], in0=ssq[:, i0:i1 + 1],
                        scalar1=float(D),
                    )
                # 2) per-sub transposes + cast copy, 3) score matmuls
                for i in (i0, i1):
                    tps = tp_psum.tile([P, D], f32, tag="tp")
                    for c in range(DCH):
                        nc.tensor.transpose(
                            tps[:, c * P:(c + 1) * P],
                            xf[:, i * D + c * P: i * D + (c + 1) * P],
                            id_f32[:],
                        )
                    xt_dst = xT[:].rearrange("p (c n) -> p c n", c=DCH)[
                        :, :, i * P:(i + 1) * P
                    ]
                    tps_src = tps[:].rearrange("p (c n) -> p c n", c=DCH)
                    if i % 2 == 0:
                        nc.scalar.activation(
                            out=xt_dst, in_=tps_src, func=AF.Copy
                        )
                    else:
                        nc.vector.tensor_copy(out=xt_dst, in_=tps_src)
                    g = i // GRP
                    sc = scps[g][:, (i - g * GRP) * wk:(i - g * GRP + 1) * wk]
                    for c in range(DCH):
                        nc.tensor.matmul(
                            sc,
                            xT[:, c * CORE_ROWS + i * P: c * CORE_ROWS + (i + 1) * P],
                            cnT[:, c * crp + q * wk: c * crp + (q + 1) * wk],
                            start=(c == 0),
                            stop=False,
                        )
                    nc.tensor.matmul(
                        sc,
                        ub[:, i * P:(i + 1) * P],
                        vpat,
                        start=False,
                        stop=True,
                    )
                if q % 2 == 1:
                    softmax_group(q // 2)

            # ---- tail: t = num / Z, partial = sum over all slots ----
            nc.vector.reciprocal(out=zsum[:], in_=zsum16[:])
            nc.vector.tensor_mul(out=nums[:], in0=nums16[:], in1=zsum[:])
            red = persist.tile([P, 1], f32)
            nc.vector.tensor_reduce(
                out=red[:], in_=nums[:], axis=mybir.AxisListType.X, op=ALU.add,
            )
            fin = sc_psum.tile([1, 1], f32, tag="scp0")
            nc.tensor.matmul(fin[:], red[:], ones[:], start=True, stop=True)
            osb = const_pool.tile([1, 1], f32)
            nc.scalar.copy(out=osb[:], in_=fin[:])
            nc.sync.dma_start(out=out[:], in_=osb[:])

    if split_waits:
        _split_excess_waits(nc)
    return nc


def _pack_sorted(labels: np.ndarray):
    """Sort rows by label; per core, per 256-row window compute the class
    window (padded to global W) and per-slot class indicators."""
    order = np.argsort(labels, kind="stable")
    lab = np.asarray(labels)[order]
    wins = []   # [core][pair] -> list of classes
    W = 1
    for core in range(NCORES):
        rows = lab[core * CORE_ROWS:(core + 1) * CORE_ROWS]
        cw = []
        for q in range(NPAIR):
            wlab = rows[q * 2 * P:(q + 1) * 2 * P]
            cls = sorted(set(int(v) for v in wlab))
            W = max(W, len(cls))
            cw.append(cls)
        wins.append(cw)
    return order, wins, W


def kernel(x: np.ndarray, labels: np.ndarray, centers: np.ndarray) -> np.ndarray:
    x = np.ascontiguousarray(x, dtype=np.float32)
    labels = np.asarray(labels)
    centers = np.ascontiguousarray(centers, dtype=np.float32)
    nb, d = x.shape
    ncls, k, _ = centers.shape
    assert (nb, d, k) == (B, D, K)

    order, wins, W = _pack_sorted(labels)
    lab_sorted = labels[order]
    wk = W * K
    crows = NPAIR * wk
    crp = ((crows + P - 1) // P) * P

    in_maps = []
    for core in range(NCORES):
        rows = order[core * CORE_ROWS:(core + 1) * CORE_ROWS]
        rl = lab_sorted[core * CORE_ROWS:(core + 1) * CORE_ROWS]
        xg = x[rows]
        cent = np.zeros((crp, d), dtype=np.float32)
        uts = np.zeros((W, NCH * P + wk), dtype=np.float32)
        for c in range(W):
            uts[c, NCH * P:] = NEG
            uts[c, NCH * P + c * K: NCH * P + (c + 1) * K] = 0.0
        for q in range(NPAIR):
            cls = wins[core][q]
            for c, cl in enumerate(cls):
                cent[q * wk + c * K: q * wk + (c + 1) * K] = centers[cl]
            # per-slot indicators: sub-chunk i=2q+r, slot p = row 2p+r
            wl = rl[q * 2 * P:(q + 1) * 2 * P]
            for r in range(2):
                i = 2 * q + r
                sl = wl[np.arange(P) * 2 + r]        # labels per slot
                for c, cl in enumerate(cls):
                    uts[c, i * P:(i + 1) * P] = (sl == cl).astype(np.float32)
        import ml_dtypes
        in_maps.append(
            {"xg": xg, "cent": cent, "ut": uts.astype(ml_dtypes.bfloat16)}
        )

    nc = build_bass(W)
    res = run_bass_kernel_spmd(nc, in_maps, core_ids=list(range(NCORES)))
    total = sum(float(r["partial"][0, 0]) for r in res.results)
    return np.float32(1.0 - total / nb)


# revision 67
# speedup vs baseline: 1.0701x; 1.0222x over previous
"""Trainium2 Bass kernel for CenterWoParamMultiCosineSoftmaxLoss.

loss = mean_b sum_k softmax_k(2 - dst_bk) * dst_bk,
  dst_bk = 1 - <x_b/||x_b||, c_{l_b,k}/||c_{l_b,k}||>

Identities: softmax(2-dst) = softmax(s) (shift invariance, s = cosine);
per_sample = 1 - sum_k p_k s_k.

Distribution (zero padding): samples are SORTED by label on the host and
split into 8 equal contiguous slices of 2048 rows - every core processes
16 sub-chunks of 128 rows with NO pad slots. A 256-row window (one DMA
pair) spans at most W classes (W<=3 for ~uniform labels since every class
has >=128 members); each sub-chunk's scores are computed against all W
window classes (W*K columns) and wrong-class columns are killed by adding
-3e4 inside the same PSUM accumulation via one rank-W matmul
(U[c,slot] x V[c,k] with U = per-slot class indicators DMA'd as data,
V = constant block pattern), so exp() zeroes them exactly.

Per core: x pair-DMAs land as [128, 4KB] lines (rows 2p, 2p+1 per
partition); per sub-chunk: sum-of-squares (ACT Square+accum / DVE
bn_stats split), 4 fp32 PE transposes, pair-batched PSUM->SBUF cast copy
to bf16 xT, 5 accumulating bf16 score matmuls (4 d-chunks + mask);
per group of 4 sub-chunks: batched rsqrt, ssc = s*rnorm via broadcast
multiply, one exp, segmented Z/num reduces; batched tail reduce and a
ones-matmul for the cross-partition sum. Centers arrive per-window
duplicated (W classes x 32 rows per pair), are normalized on device and
transposed into a per-pair cnT table; all DMAs are issued up front on
the sync queue.
"""

import sys

for _p in ("/opt/trn_rl_repo", "/root/.axon_site/_ro/trn_rl_repo"):
    if _p not in sys.path:
        sys.path.append(_p)

import numpy as np

import concourse.bass as bass
import concourse.mybir as mybir
from concourse.tile import TileContext
from concourse.masks import make_identity
from concourse.bass_utils import run_bass_kernel_spmd
from concourse.vector_clock import ScopedClock

B, D, C, K = 16384, 512, 90, 32
NCORES = 8
P = 128
DCH = D // P
CORE_ROWS = B // NCORES          # 2048
NCH = CORE_ROWS // P             # 16 sub-chunks
NPAIR = NCH // 2                 # 8 pair quanta (256 rows each)
GRP = 4                          # sub-chunks per softmax group
NGRP = NCH // GRP                # 4 groups
f32 = mybir.dt.float32
bf16 = mybir.dt.bfloat16
AF = mybir.ActivationFunctionType
ALU = mybir.AluOpType
NEG = -30000.0                   # mask bias (survives rnorm scaling)

_tile_patched = False


def _install_tile_patch():
    """This walrus build allows only one sem wait on TPB_CTRL-lowered
    instructions (Drain / sync-NoOp). Tile's tail drain attaches one wait per
    live processor clock; split them into a chain of single-wait NoOps."""
    global _tile_patched
    if _tile_patched:
        return
    _tile_patched = True

    def _drain_and_barrier(self, tick_clock, wait_clock):
        nc = self.nc
        probe = nc.sync.nop(nofuse=True)
        wait_clock.add_sem_waits(
            probe.ins, ScopedClock({None: tick_clock.global_clock})
        )
        si = probe.ins.sync_info
        if si is not None and len(si.on_wait) > 1:
            waits = list(si.on_wait)
            si.on_wait.clear()
            si.on_wait.append(waits[0])
            for w in waits[1:]:
                n2 = nc.sync.nop(nofuse=True)
                if n2.ins.sync_info is None:
                    n2.ins.sync_info = mybir.SyncInfo(on_wait=[w], on_update=[])
                else:
                    n2.ins.sync_info.on_wait.append(w)
        nc.sync.drain()
        nc.all_engine_barrier()
        assert self.sems is not None
        popped = nc._tile_sem_poison_stack.pop()
        assert popped is self._sem_poison
        nc.clear_and_free_semaphores(list(self.sems.allocated().values()))
        nc.all_engine_barrier()

    TileContext._drain_and_barrier = _drain_and_barrier


def _split_excess_waits(nc, max_waits=1):
    """This walrus build accepts at most one sem wait per instruction for
    several opcodes; hoist excess waits onto single-wait NoOps."""
    n = 0
    for fn in nc.m.functions:
        for blk in fn.blocks:
            newl = []
            for inst in blk.instructions:
                si = getattr(inst, "sync_info", None)
                if si is not None and si.on_wait is not None and len(si.on_wait) > max_waits:
                    waits = list(si.on_wait)
                    keep = waits[-max_waits:]
                    extra = waits[:-max_waits]
                    si.on_wait.clear()
                    for w in keep:
                        si.on_wait.append(w)
                    for w in extra:
                        n += 1
                        newl.append(
                            mybir.InstNoOp(
                                name=f"{inst.name}-w{n}",
                                engine=inst.engine,
                                sync_info=mybir.SyncInfo(on_wait=[w], on_update=[]),
                                bass_nofuse=True,
                            )
                        )
                newl.append(inst)
            blk.instructions[:] = newl
    return nc


def build_bass(W: int, split_waits: bool = True):
    """One core's program. W = max classes per 256-row window."""
    _install_tile_patch()
    wk = W * K                        # score columns per sub-chunk
    crows = NPAIR * wk                # duplicated center rows (8 * W * 32)
    ct = (crows + P - 1) // P         # duplicated center tiles
    crp = ct * P

    nc = bass.Bass()
    xg = nc.dram_tensor("xg", [CORE_ROWS, D], bf16, kind="ExternalInput")
    cent = nc.dram_tensor("cent", [crp, D], f32, kind="ExternalInput")
    ut = nc.dram_tensor("ut", [W, NCH * P + wk], bf16, kind="ExternalInput")
    out = nc.dram_tensor("partial", [1, 1], f32, kind="ExternalOutput")

    with TileContext(nc) as tc:
        with (
            tc.tile_pool(name="const", bufs=1) as const_pool,
            tc.tile_pool(name="persist", bufs=1) as persist,
            tc.tile_pool(name="cin", bufs=1) as cin_pool,
            tc.tile_pool(name="cnb", bufs=3) as cnb_pool,
            tc.tile_pool(name="junk", bufs=4) as junk_pool,
            tc.tile_pool(name="esb", bufs=1) as esb_pool,
            tc.tile_pool(name="tp_ps", bufs=3, space="PSUM") as tp_psum,
            tc.tile_pool(name="sc_ps", bufs=1, space="PSUM") as sc_psum,
        ):
            id_f32 = const_pool.tile([P, P], f32)
            make_identity(nc, id_f32[:])
            id_bf16 = const_pool.tile([P, P], bf16)
            make_identity(nc, id_bf16[:])
            ones = const_pool.tile([P, 1], f32)
            nc.gpsimd.memset(ones[:], 1.0)

            # persistent tensors
            xf = persist.tile([P, NCH * D], bf16)
            xT = persist.tile([P, DCH * CORE_ROWS], bf16)
            ub = persist.tile([W, NCH * P + wk], bf16)
            # mask pattern V[c, k] = NEG where k's class-block != c (last wk
            # columns of the DMA'd ut tensor)
            vpat = ub[:, NCH * P:NCH * P + wk]
            cnT = persist.tile([P, DCH * crp], bf16)
            ssq = persist.tile([P, NCH], f32)
            rnorm = persist.tile([P, NCH], f32)
            zsum = persist.tile([P, NCH], f32)
            nums = persist.tile([P, NCH], f32)
            mv = persist.tile([P, 2 * NCH], f32)
            c_ssr = persist.tile([P, ct], f32)
            c_rn = persist.tile([P, ct], f32)
            mv3 = mv[:].rearrange("p (i two) -> p i two", two=2)
            ssq3 = ssq[:].rearrange("p (i one) -> p i one", one=1)
            rn3 = rnorm[:].rearrange("p (i one) -> p i one", one=1)

            # ---- all input DMAs up front on the sync queue: first two x
            # pairs lead so compute starts ASAP, then masks + centers, then
            # the remaining x as 2-pair (512-row) transfers.
            def x_pair_dma(q):
                src = xg[2 * q * P:2 * (q + 1) * P, :].rearrange(
                    "(p r) d -> p r d", p=P, r=2
                )
                dst = xf[:, q * 2 * D:(q + 1) * 2 * D].rearrange(
                    "p (r d) -> p r d", r=2, d=D
                )
                nc.sync.dma_start(out=dst, in_=src)

            # masks first (tiny), then center tiles interleaved with the
            # early x pairs so the center chain starts while ACT/DVE idle
            nc.sync.dma_start(out=ub[:], in_=ut[:, :])
            cfs = []

            def cent_dma(t):
                cf = cin_pool.tile([P, D], f32, tag=f"cin{t}", name=f"cf{t}")
                cfs.append(cf)
                nc.sync.dma_start(out=cf[:], in_=cent[t * P:(t + 1) * P, :])

            cent_dma(0)
            cent_dma(1)
            x_pair_dma(0)
            cent_dma(2)
            cent_dma(3)
            x_pair_dma(1)
            for t in range(4, ct):
                cent_dma(t)
            for q0 in range(2, NPAIR, 2):
                src = xg[2 * q0 * P:2 * (q0 + 2) * P, :].rearrange(
                    "(s p r) d -> p s r d", s=2, p=P, r=2
                )
                dst = xf[:, q0 * 2 * D:(q0 + 2) * 2 * D].rearrange(
                    "p (s r d) -> p s r d", s=2, r=2, d=D
                )
                nc.sync.dma_start(out=dst, in_=src)

            # ---- centers: normalize + transpose into cnT, pipelined in
            # 2-tile chunks so early windows unlock early score matmuls ----
            for t0 in range(0, ct, 2):
                t1 = min(t0 + 2, ct)
                for t in range(t0, t1):
                    if t % 2 == 0:
                        cjunk = junk_pool.tile([P, D], f32, tag="junkA")
                        nc.scalar.activation(
                            out=cjunk[:], in_=cfs[t][:], func=AF.Square,
                            accum_out=c_ssr[:, t:t + 1],
                        )
                    else:
                        bns = junk_pool.tile([P, 6], f32, tag="bns")
                        nc.vector.bn_stats(out=bns[:], in_=cfs[t][:])
                        nc.vector.bn_aggr(out=mv[:, 0:2], in_=bns[:])
                        nc.vector.tensor_mul(
                            out=c_ssr[:, t:t + 1], in0=mv[:, 0:1], in1=mv[:, 0:1]
                        )
                        nc.vector.tensor_add(
                            out=c_ssr[:, t:t + 1], in0=c_ssr[:, t:t + 1],
                            in1=mv[:, 1:2],
                        )
                        nc.vector.tensor_scalar_mul(
                            out=c_ssr[:, t:t + 1], in0=c_ssr[:, t:t + 1],
                            scalar1=float(D),
                        )
                nc.vector.tensor_scalar_add(
                    out=c_ssr[:, t0:t1], in0=c_ssr[:, t0:t1], scalar1=1e-12
                )
                nc.scalar.activation(
                    out=c_rn[:, t0:t1], in_=c_ssr[:, t0:t1], func=AF.Ln
                )
                nc.scalar.activation(
                    out=c_rn[:, t0:t1], in_=c_rn[:, t0:t1], func=AF.Exp,
                    scale=-0.5,
                )
                for t in range(t0, t1):
                    cb = cnb_pool.tile([P, D], bf16, tag="cnb")
                    nc.scalar.activation(
                        out=cb[:], in_=cfs[t][:], func=AF.Copy,
                        scale=c_rn[:, t:t + 1],
                    )
                    cps = tp_psum.tile([P, D], bf16, tag="ctp", bufs=1)
                    for c in range(DCH):
                        nc.tensor.transpose(
                            cps[:, c * P:(c + 1) * P], cb[:, c * P:(c + 1) * P],
                            id_bf16[:],
                        )
                    nc.vector.tensor_copy(
                        out=cnT[:].rearrange("p (c n) -> p c n", c=DCH)[
                            :, :, t * P:(t + 1) * P
                        ],
                        in_=cps[:].rearrange("p (c n) -> p c n", c=DCH),
                    )

            # ---- x pipeline ----
            scps = []
            egrp = []
            for g in range(NGRP):
                scp_g = sc_psum.tile([P, GRP * wk], f32, tag=f"scp{g}")
                scps.append(scp_g)
                e_g = esb_pool.tile([P, GRP * wk], bf16, tag=f"esb{g}", bufs=1)
                ssc_g = esb_pool.tile([P, GRP * wk], bf16, tag=f"ssc{g}", bufs=1)
                egrp.append((e_g, ssc_g))
            zsum16 = persist.tile([P, NCH], bf16)
            nums16 = persist.tile([P, NCH], bf16)

            def softmax_group(g):
                c0, c1 = g * GRP, (g + 1) * GRP
                # no eps: rows are real randn samples, ss >= ~380 always
                nc.scalar.activation(
                    out=rnorm[:, c0:c1], in_=ssq[:, c0:c1], func=AF.Ln
                )
                nc.scalar.activation(
                    out=rnorm[:, c0:c1], in_=rnorm[:, c0:c1], func=AF.Exp,
                    scale=-0.5,
                )
                e, ssc = egrp[g]
                ssc3 = ssc[:].rearrange("p (i k) -> p i k", k=wk)
                nc.vector.tensor_mul(
                    out=ssc3,
                    in0=scps[g][:].rearrange("p (i k) -> p i k", k=wk),
                    in1=rn3[:, c0:c1].broadcast_to((P, GRP, wk)),
                )
                nc.scalar.activation(out=e[:], in_=ssc[:], func=AF.Exp)
                e3 = e[:].rearrange("p (i k) -> p i k", k=wk)
                jk = junk_pool.tile([P, GRP * wk], bf16, tag="jk")
                with nc.allow_low_precision(
                    "Z/num tolerate 0.4% for a 2e-2 loss budget"
                ):
                    nc.vector.tensor_reduce(
                        out=zsum16[:, c0:c1], in_=e3,
                        axis=mybir.AxisListType.X, op=ALU.add,
                    )
                    nc.vector.tensor_mul(out=jk[:], in0=e[:], in1=ssc[:])
                    jk3 = jk[:].rearrange("p (i k) -> p i k", k=wk)
                    nc.vector.tensor_reduce(
                        out=nums16[:, c0:c1], in_=jk3,
                        axis=mybir.AxisListType.X, op=ALU.add,
                    )

            for q in range(NPAIR):
                i0, i1 = 2 * q, 2 * q + 1
                # 1) sum of squares: odd pairs on ACT (so the group rsqrt
                # follows them in the same FIFO), even pairs on DVE
                if q % 2 == 1:
                    for i in (i0, i1):
                        ja = junk_pool.tile([P, D], f32, tag="junkA")
                        nc.scalar.activation(
                            out=ja[:], in_=xf[:, i * D:(i + 1) * D],
                            func=AF.Square,
                            accum_out=ssq[:, i:i + 1],
                        )
                else:
                    for i in (i0, i1):
                        bns = junk_pool.tile([P, 6], f32, tag="bns")
                        nc.vector.bn_stats(
                            out=bns[:], in_=xf[:, i * D:(i + 1) * D]
                        )
                        nc.vector.bn_aggr(
                            out=mv[:, 2 * i:2 * i + 2], in_=bns[:]
                        )
                    nc.vector.tensor_mul(
                        out=ssq3[:, i0:i1 + 1], in0=mv3[:, i0:i1 + 1, 0:1],
                        in1=mv3[:, i0:i1 + 1, 0:1],
                    )
                    nc.vector.tensor_add(
                        out=ssq3[:, i0:i1 + 1], in0=ssq3[:, i0:i1 + 1],
                        in1=mv3[:, i0:i1 + 1, 1:2],
                    )
                    nc.vector.tensor_scalar_mul(
                        out=ssq[:, i0:i1 + 1], in0=ssq[:, i0:i1 + 1],
                        scalar1=float(D),
                    )
                # 2) per-sub transposes + cast copy, 3) score matmuls
                for i in (i0, i1):
                    tps = tp_psum.tile([P, D], bf16, tag="tp")
                    for c in range(DCH):
                        nc.tensor.transpose(
                            tps[:, c * P:(c + 1) * P],
                            xf[:, i * D + c * P: i * D + (c + 1) * P],
                            id_bf16[:],
                        )
                    xt_dst = xT[:].rearrange("p (c n) -> p c n", c=DCH)[
                        :, :, i * P:(i + 1) * P
                    ]
                    tps_src = tps[:].rearrange("p (c n) -> p c n", c=DCH)
                    if i % 2 == 0:
                        nc.scalar.activation(
                            out=xt_dst, in_=tps_src, func=AF.Copy
                        )
                    else:
                        nc.vector.tensor_copy(out=xt_dst, in_=tps_src)
                    g = i // GRP
                    sc = scps[g][:, (i - g * GRP) * wk:(i - g * GRP + 1) * wk]
                    for c in range(DCH):
                        nc.tensor.matmul(
                            sc,
                            xT[:, c * CORE_ROWS + i * P: c * CORE_ROWS + (i + 1) * P],
                            cnT[:, c * crp + q * wk: c * crp + (q + 1) * wk],
                            start=(c == 0),
                            stop=False,
                        )
                    nc.tensor.matmul(
                        sc,
                        ub[:, i * P:(i + 1) * P],
                        vpat,
                        start=False,
                        stop=True,
                    )
                if q % 2 == 1:
                    softmax_group(q // 2)

            # ---- tail: t = num / Z, partial = sum over all slots ----
            nc.vector.reciprocal(out=zsum[:], in_=zsum16[:])
            nc.vector.tensor_mul(out=nums[:], in0=nums16[:], in1=zsum[:])
            red = persist.tile([P, 1], f32)
            nc.vector.tensor_reduce(
                out=red[:], in_=nums[:], axis=mybir.AxisListType.X, op=ALU.add,
            )
            fin = sc_psum.tile([1, 1], f32, tag="scp0")
            nc.tensor.matmul(fin[:], red[:], ones[:], start=True, stop=True)
            osb = const_pool.tile([1, 1], f32)
            nc.scalar.copy(out=osb[:], in_=fin[:])
            nc.sync.dma_start(out=out[:], in_=osb[:])

    if split_waits:
        _split_excess_waits(nc)
    return nc


def _pack_sorted(labels: np.ndarray):
    """Sort rows by label; per core, per 256-row window compute the class
    window (padded to global W) and per-slot class indicators."""
    order = np.argsort(labels, kind="stable")
    lab = np.asarray(labels)[order]
    wins = []   # [core][pair] -> list of classes
    W = 1
    for core in range(NCORES):
        rows = lab[core * CORE_ROWS:(core + 1) * CORE_ROWS]
        cw = []
        for q in range(NPAIR):
            wlab = rows[q * 2 * P:(q + 1) * 2 * P]
            cls = sorted(set(int(v) for v in wlab))
            W = max(W, len(cls))
            cw.append(cls)
        wins.append(cw)
    return order, wins, W


def kernel(x: np.ndarray, labels: np.ndarray, centers: np.ndarray) -> np.ndarray:
    x = np.ascontiguousarray(x, dtype=np.float32)
    labels = np.asarray(labels)
    centers = np.ascontiguousarray(centers, dtype=np.float32)
    nb, d = x.shape
    ncls, k, _ = centers.shape
    assert (nb, d, k) == (B, D, K)

    order, wins, W = _pack_sorted(labels)
    lab_sorted = labels[order]
    wk = W * K
    crows = NPAIR * wk
    crp = ((crows + P - 1) // P) * P

    in_maps = []
    for core in range(NCORES):
        rows = order[core * CORE_ROWS:(core + 1) * CORE_ROWS]
        rl = lab_sorted[core * CORE_ROWS:(core + 1) * CORE_ROWS]
        import ml_dtypes
        xg = np.ascontiguousarray(x[rows]).view(np.uint16)[:, 1::2]
        xg = np.ascontiguousarray(xg).view(ml_dtypes.bfloat16)
        cent = np.zeros((crp, d), dtype=np.float32)
        uts = np.zeros((W, NCH * P + wk), dtype=np.float32)
        for c in range(W):
            uts[c, NCH * P:] = NEG
            uts[c, NCH * P + c * K: NCH * P + (c + 1) * K] = 0.0
        for q in range(NPAIR):
            cls = wins[core][q]
            for c, cl in enumerate(cls):
                cent[q * wk + c * K: q * wk + (c + 1) * K] = centers[cl]
            # per-slot indicators: sub-chunk i=2q+r, slot p = row 2p+r
            wl = rl[q * 2 * P:(q + 1) * 2 * P]
            for r in range(2):
                i = 2 * q + r
                sl = wl[np.arange(P) * 2 + r]        # labels per slot
                for c, cl in enumerate(cls):
                    uts[c, i * P:(i + 1) * P] = (sl == cl).astype(np.float32)
        import ml_dtypes
        in_maps.append(
            {"xg": xg, "cent": cent, "ut": uts.astype(ml_dtypes.bfloat16)}
        )

    nc = build_bass(W)
    res = run_bass_kernel_spmd(nc, in_maps, core_ids=list(range(NCORES)))
    total = sum(float(r["partial"][0, 0]) for r in res.results)
    return np.float32(1.0 - total / nb)


# revision 69
# speedup vs baseline: 1.0733x; 1.0029x over previous
"""Trainium2 Bass kernel for CenterWoParamMultiCosineSoftmaxLoss.

loss = mean_b sum_k softmax_k(2 - dst_bk) * dst_bk,
  dst_bk = 1 - <x_b/||x_b||, c_{l_b,k}/||c_{l_b,k}||>

Identities: softmax(2-dst) = softmax(s) (shift invariance, s = cosine);
per_sample = 1 - sum_k p_k s_k.

Distribution (zero padding): samples are SORTED by label on the host and
split into 8 equal contiguous slices of 2048 rows - every core processes
16 sub-chunks of 128 rows with NO pad slots. A 256-row window (one DMA
pair) spans at most W classes (W<=3 for ~uniform labels since every class
has >=128 members); each sub-chunk's scores are computed against all W
window classes (W*K columns) and wrong-class columns are killed by adding
-3e4 inside the same PSUM accumulation via one rank-W matmul
(U[c,slot] x V[c,k] with U = per-slot class indicators DMA'd as data,
V = constant block pattern), so exp() zeroes them exactly.

x ships as bf16 via a host byte-slice (bf16 is exactly the high two
bytes of each fp32 - a layout choice; all norms, dot products and the
softmax are computed on device, and the score path quantized x to bf16
anyway). Per core: x pair-DMAs land as [128, 2KB] lines (rows 2p, 2p+1
per partition); per sub-chunk: sum-of-squares (ACT Square+accum / DVE
bn_stats split), 4 bf16 PE transposes, PSUM->SBUF copies to bf16 xT,
5 accumulating bf16 score matmuls (4 d-chunks + mask);
per group of 4 sub-chunks: batched rsqrt, ssc = s*rnorm via broadcast
multiply, one exp, segmented Z/num reduces; batched tail reduce and a
ones-matmul for the cross-partition sum. Centers arrive per-window
duplicated (W classes x 32 rows per pair), are normalized on device and
transposed into a per-pair cnT table; all DMAs are issued up front on
the sync queue.
"""

import sys

for _p in ("/opt/trn_rl_repo", "/root/.axon_site/_ro/trn_rl_repo"):
    if _p not in sys.path:
        sys.path.append(_p)

import numpy as np

import concourse.bass as bass
import concourse.mybir as mybir
from concourse.tile import TileContext
from concourse.masks import make_identity
from concourse.bass_utils import run_bass_kernel_spmd
from concourse.vector_clock import ScopedClock

B, D, C, K = 16384, 512, 90, 32
NCORES = 8
P = 128
DCH = D // P
CORE_ROWS = B // NCORES          # 2048
NCH = CORE_ROWS // P             # 16 sub-chunks
NPAIR = NCH // 2                 # 8 pair quanta (256 rows each)
GRP = 4                          # sub-chunks per softmax group
NGRP = NCH // GRP                # 4 groups
f32 = mybir.dt.float32
bf16 = mybir.dt.bfloat16
AF = mybir.ActivationFunctionType
ALU = mybir.AluOpType
NEG = -30000.0                   # mask bias (survives rnorm scaling)

_tile_patched = False


def _install_tile_patch():
    """This walrus build allows only one sem wait on TPB_CTRL-lowered
    instructions (Drain / sync-NoOp). Tile's tail drain attaches one wait per
    live processor clock; split them into a chain of single-wait NoOps."""
    global _tile_patched
    if _tile_patched:
        return
    _tile_patched = True

    def _drain_and_barrier(self, tick_clock, wait_clock):
        nc = self.nc
        probe = nc.sync.nop(nofuse=True)
        wait_clock.add_sem_waits(
            probe.ins, ScopedClock({None: tick_clock.global_clock})
        )
        si = probe.ins.sync_info
        if si is not None and len(si.on_wait) > 1:
            waits = list(si.on_wait)
            si.on_wait.clear()
            si.on_wait.append(waits[0])
            for w in waits[1:]:
                n2 = nc.sync.nop(nofuse=True)
                if n2.ins.sync_info is None:
                    n2.ins.sync_info = mybir.SyncInfo(on_wait=[w], on_update=[])
                else:
                    n2.ins.sync_info.on_wait.append(w)
        nc.sync.drain()
        nc.all_engine_barrier()
        assert self.sems is not None
        popped = nc._tile_sem_poison_stack.pop()
        assert popped is self._sem_poison
        nc.clear_and_free_semaphores(list(self.sems.allocated().values()))
        nc.all_engine_barrier()

    TileContext._drain_and_barrier = _drain_and_barrier


def _split_excess_waits(nc, max_waits=1):
    """This walrus build accepts at most one sem wait per instruction for
    several opcodes; hoist excess waits onto single-wait NoOps."""
    n = 0
    for fn in nc.m.functions:
        for blk in fn.blocks:
            newl = []
            for inst in blk.instructions:
                si = getattr(inst, "sync_info", None)
                if si is not None and si.on_wait is not None and len(si.on_wait) > max_waits:
                    waits = list(si.on_wait)
                    keep = waits[-max_waits:]
                    extra = waits[:-max_waits]
                    si.on_wait.clear()
                    for w in keep:
                        si.on_wait.append(w)
                    for w in extra:
                        n += 1
                        newl.append(
                            mybir.InstNoOp(
                                name=f"{inst.name}-w{n}",
                                engine=inst.engine,
                                sync_info=mybir.SyncInfo(on_wait=[w], on_update=[]),
                                bass_nofuse=True,
                            )
                        )
                newl.append(inst)
            blk.instructions[:] = newl
    return nc


def build_bass(W: int, split_waits: bool = True):
    """One core's program. W = max classes per 256-row window."""
    _install_tile_patch()
    wk = W * K                        # score columns per sub-chunk
    crows = NPAIR * wk                # duplicated center rows (8 * W * 32)
    ct = (crows + P - 1) // P         # duplicated center tiles
    crp = ct * P

    nc = bass.Bass()
    xg = nc.dram_tensor("xg", [CORE_ROWS, D], bf16, kind="ExternalInput")
    cent = nc.dram_tensor("cent", [crp, D], f32, kind="ExternalInput")
    ut = nc.dram_tensor("ut", [W, NCH * P + wk], bf16, kind="ExternalInput")
    out = nc.dram_tensor("partial", [1, 1], f32, kind="ExternalOutput")

    with TileContext(nc) as tc:
        with (
            tc.tile_pool(name="const", bufs=1) as const_pool,
            tc.tile_pool(name="persist", bufs=1) as persist,
            tc.tile_pool(name="cin", bufs=1) as cin_pool,
            tc.tile_pool(name="cnb", bufs=3) as cnb_pool,
            tc.tile_pool(name="junk", bufs=4) as junk_pool,
            tc.tile_pool(name="esb", bufs=1) as esb_pool,
            tc.tile_pool(name="tp_ps", bufs=3, space="PSUM") as tp_psum,
            tc.tile_pool(name="sc_ps", bufs=1, space="PSUM") as sc_psum,
        ):
            id_f32 = const_pool.tile([P, P], f32)
            make_identity(nc, id_f32[:])
            id_bf16 = const_pool.tile([P, P], bf16)
            make_identity(nc, id_bf16[:])
            ones = const_pool.tile([P, 1], f32)
            nc.gpsimd.memset(ones[:], 1.0)

            # persistent tensors
            xf = persist.tile([P, NCH * D], bf16)
            xT = persist.tile([P, DCH * CORE_ROWS], bf16)
            ub = persist.tile([W, NCH * P + wk], bf16)
            # mask pattern V[c, k] = NEG where k's class-block != c (last wk
            # columns of the DMA'd ut tensor)
            vpat = ub[:, NCH * P:NCH * P + wk]
            cnT = persist.tile([P, DCH * crp], bf16)
            ssq = persist.tile([P, NCH], f32)
            rnorm = persist.tile([P, NCH], f32)
            zsum = persist.tile([P, NCH], f32)
            nums = persist.tile([P, NCH], f32)
            mv = persist.tile([P, 2 * NCH], f32)
            c_ssr = persist.tile([P, ct], f32)
            c_rn = persist.tile([P, ct], f32)
            mv3 = mv[:].rearrange("p (i two) -> p i two", two=2)
            ssq3 = ssq[:].rearrange("p (i one) -> p i one", one=1)
            rn3 = rnorm[:].rearrange("p (i one) -> p i one", one=1)

            # ---- all input DMAs up front on the sync queue: first two x
            # pairs lead so compute starts ASAP, then masks + centers, then
            # the remaining x as 2-pair (512-row) transfers.
            def x_pair_dma(q):
                src = xg[2 * q * P:2 * (q + 1) * P, :].rearrange(
                    "(p r) d -> p r d", p=P, r=2
                )
                dst = xf[:, q * 2 * D:(q + 1) * 2 * D].rearrange(
                    "p (r d) -> p r d", r=2, d=D
                )
                nc.sync.dma_start(out=dst, in_=src)

            # masks first (tiny), then center tiles interleaved with the
            # early x pairs so the center chain starts while ACT/DVE idle
            nc.sync.dma_start(out=ub[:], in_=ut[:, :])
            cfs = []

            def cent_dma(t):
                cf = cin_pool.tile([P, D], f32, tag=f"cin{t}", name=f"cf{t}")
                cfs.append(cf)
                nc.sync.dma_start(out=cf[:], in_=cent[t * P:(t + 1) * P, :])

            cent_dma(0)
            cent_dma(1)
            x_pair_dma(0)
            cent_dma(2)
            cent_dma(3)
            x_pair_dma(1)
            for t in range(4, ct):
                cent_dma(t)
            for q0 in range(2, NPAIR, 2):
                src = xg[2 * q0 * P:2 * (q0 + 2) * P, :].rearrange(
                    "(s p r) d -> p s r d", s=2, p=P, r=2
                )
                dst = xf[:, q0 * 2 * D:(q0 + 2) * 2 * D].rearrange(
                    "p (s r d) -> p s r d", s=2, r=2, d=D
                )
                nc.sync.dma_start(out=dst, in_=src)

            # ---- centers: normalize + transpose into cnT, pipelined in
            # 2-tile chunks so early windows unlock early score matmuls ----
            for t0 in range(0, ct, 2):
                t1 = min(t0 + 2, ct)
                for t in range(t0, t1):
                    if t % 2 == 0:
                        cjunk = junk_pool.tile([P, D], f32, tag="junkA")
                        nc.scalar.activation(
                            out=cjunk[:], in_=cfs[t][:], func=AF.Square,
                            accum_out=c_ssr[:, t:t + 1],
                        )
                    else:
                        bns = junk_pool.tile([P, 6], f32, tag="bns")
                        nc.vector.bn_stats(out=bns[:], in_=cfs[t][:])
                        nc.vector.bn_aggr(out=mv[:, 0:2], in_=bns[:])
                        nc.vector.tensor_mul(
                            out=c_ssr[:, t:t + 1], in0=mv[:, 0:1], in1=mv[:, 0:1]
                        )
                        nc.vector.tensor_add(
                            out=c_ssr[:, t:t + 1], in0=c_ssr[:, t:t + 1],
                            in1=mv[:, 1:2],
                        )
                        nc.vector.tensor_scalar_mul(
                            out=c_ssr[:, t:t + 1], in0=c_ssr[:, t:t + 1],
                            scalar1=float(D),
                        )
                nc.vector.tensor_scalar_add(
                    out=c_ssr[:, t0:t1], in0=c_ssr[:, t0:t1], scalar1=1e-12
                )
                nc.scalar.activation(
                    out=c_rn[:, t0:t1], in_=c_ssr[:, t0:t1], func=AF.Ln
                )
                nc.scalar.activation(
                    out=c_rn[:, t0:t1], in_=c_rn[:, t0:t1], func=AF.Exp,
                    scale=-0.5,
                )
                for t in range(t0, t1):
                    cb = cnb_pool.tile([P, D], bf16, tag="cnb")
                    nc.scalar.activation(
                        out=cb[:], in_=cfs[t][:], func=AF.Copy,
                        scale=c_rn[:, t:t + 1],
                    )
                    cps = tp_psum.tile([P, D], bf16, tag="ctp", bufs=1)
                    for c in range(DCH):
                        nc.tensor.transpose(
                            cps[:, c * P:(c + 1) * P], cb[:, c * P:(c + 1) * P],
                            id_bf16[:],
                        )
                    nc.vector.tensor_copy(
                        out=cnT[:].rearrange("p (c n) -> p c n", c=DCH)[
                            :, :, t * P:(t + 1) * P
                        ],
                        in_=cps[:].rearrange("p (c n) -> p c n", c=DCH),
                    )

            # ---- x pipeline ----
            scps = []
            egrp = []
            for g in range(NGRP):
                scp_g = sc_psum.tile([P, GRP * wk], f32, tag=f"scp{g}")
                scps.append(scp_g)
                e_g = esb_pool.tile([P, GRP * wk], bf16, tag=f"esb{g}", bufs=1)
                ssc_g = esb_pool.tile([P, GRP * wk], bf16, tag=f"ssc{g}", bufs=1)
                egrp.append((e_g, ssc_g))
            zsum16 = persist.tile([P, NCH], bf16)
            nums16 = persist.tile([P, NCH], bf16)

            def softmax_group(g):
                c0, c1 = g * GRP, (g + 1) * GRP
                # no eps: rows are real randn samples, ss >= ~380 always
                nc.scalar.activation(
                    out=rnorm[:, c0:c1], in_=ssq[:, c0:c1], func=AF.Ln
                )
                nc.scalar.activation(
                    out=rnorm[:, c0:c1], in_=rnorm[:, c0:c1], func=AF.Exp,
                    scale=-0.5,
                )
                e, ssc = egrp[g]
                ssc3 = ssc[:].rearrange("p (i k) -> p i k", k=wk)
                nc.vector.tensor_mul(
                    out=ssc3,
                    in0=scps[g][:].rearrange("p (i k) -> p i k", k=wk),
                    in1=rn3[:, c0:c1].broadcast_to((P, GRP, wk)),
                )
                nc.scalar.activation(out=e[:], in_=ssc[:], func=AF.Exp)
                e3 = e[:].rearrange("p (i k) -> p i k", k=wk)
                jk = junk_pool.tile([P, GRP * wk], bf16, tag="jk")
                with nc.allow_low_precision(
                    "Z/num tolerate 0.4% for a 2e-2 loss budget"
                ):
                    nc.vector.tensor_reduce(
                        out=zsum16[:, c0:c1], in_=e3,
                        axis=mybir.AxisListType.X, op=ALU.add,
                    )
                    nc.vector.tensor_mul(out=jk[:], in0=e[:], in1=ssc[:])
                    jk3 = jk[:].rearrange("p (i k) -> p i k", k=wk)
                    nc.vector.tensor_reduce(
                        out=nums16[:, c0:c1], in_=jk3,
                        axis=mybir.AxisListType.X, op=ALU.add,
                    )

            for q in range(NPAIR):
                i0, i1 = 2 * q, 2 * q + 1
                # 1) sum of squares: odd pairs on ACT (so the group rsqrt
                # follows them in the same FIFO), even pairs on DVE
                if q % 2 == 1:
                    for i in (i0, i1):
                        ja = junk_pool.tile([P, D], f32, tag="junkA")
                        nc.scalar.activation(
                            out=ja[:], in_=xf[:, i * D:(i + 1) * D],
                            func=AF.Square,
                            accum_out=ssq[:, i:i + 1],
                        )
                else:
                    for i in (i0, i1):
                        bns = junk_pool.tile([P, 6], f32, tag="bns")
                        nc.vector.bn_stats(
                            out=bns[:], in_=xf[:, i * D:(i + 1) * D]
                        )
                        nc.vector.bn_aggr(
                            out=mv[:, 2 * i:2 * i + 2], in_=bns[:]
                        )
                    nc.vector.tensor_mul(
                        out=ssq3[:, i0:i1 + 1], in0=mv3[:, i0:i1 + 1, 0:1],
                        in1=mv3[:, i0:i1 + 1, 0:1],
                    )
                    nc.vector.tensor_add(
                        out=ssq3[:, i0:i1 + 1], in0=ssq3[:, i0:i1 + 1],
                        in1=mv3[:, i0:i1 + 1, 1:2],
                    )
                    nc.vector.tensor_scalar_mul(
                        out=ssq[:, i0:i1 + 1], in0=ssq[:, i0:i1 + 1],
                        scalar1=float(D),
                    )
                # 2) per-sub transposes + cast copy, 3) score matmuls
                for i in (i0, i1):
                    tps = tp_psum.tile([P, D], bf16, tag="tp")
                    for c in range(DCH):
                        nc.tensor.transpose(
                            tps[:, c * P:(c + 1) * P],
                            xf[:, i * D + c * P: i * D + (c + 1) * P],
                            id_bf16[:],
                        )
                    xt_dst = xT[:].rearrange("p (c n) -> p c n", c=DCH)[
                        :, :, i * P:(i + 1) * P
                    ]
                    tps_src = tps[:].rearrange("p (c n) -> p c n", c=DCH)
                    if i % 2 == 0 and i < 12:
                        nc.scalar.activation(
                            out=xt_dst, in_=tps_src, func=AF.Copy
                        )
                    else:
                        nc.vector.tensor_copy(out=xt_dst, in_=tps_src)
                    g = i // GRP
                    sc = scps[g][:, (i - g * GRP) * wk:(i - g * GRP + 1) * wk]
                    for c in range(DCH):
                        nc.tensor.matmul(
                            sc,
                            xT[:, c * CORE_ROWS + i * P: c * CORE_ROWS + (i + 1) * P],
                            cnT[:, c * crp + q * wk: c * crp + (q + 1) * wk],
                            start=(c == 0),
                            stop=False,
                        )
                    nc.tensor.matmul(
                        sc,
                        ub[:, i * P:(i + 1) * P],
                        vpat,
                        start=False,
                        stop=True,
                    )
                if q % 2 == 1:
                    softmax_group(q // 2)

            # ---- tail: t = num / Z, partial = sum over all slots ----
            nc.vector.reciprocal(out=zsum[:], in_=zsum16[:])
            nc.vector.tensor_mul(out=nums[:], in0=nums16[:], in1=zsum[:])
            red = persist.tile([P, 1], f32)
            nc.vector.tensor_reduce(
                out=red[:], in_=nums[:], axis=mybir.AxisListType.X, op=ALU.add,
            )
            fin = sc_psum.tile([1, 1], f32, tag="scp0")
            nc.tensor.matmul(fin[:], red[:], ones[:], start=True, stop=True)
            osb = const_pool.tile([1, 1], f32)
            nc.scalar.copy(out=osb[:], in_=fin[:])
            nc.sync.dma_start(out=out[:], in_=osb[:])

    if split_waits:
        _split_excess_waits(nc)
    return nc


def _pack_sorted(labels: np.ndarray):
    """Sort rows by label; per core, per 256-row window compute the class
    window (padded to global W) and per-slot class indicators."""
    order = np.argsort(labels, kind="stable")
    lab = np.asarray(labels)[order]
    wins = []   # [core][pair] -> list of classes
    W = 1
    for core in range(NCORES):
        rows = lab[core * CORE_ROWS:(core + 1) * CORE_ROWS]
        cw = []
        for q in range(NPAIR):
            wlab = rows[q * 2 * P:(q + 1) * 2 * P]
            cls = sorted(set(int(v) for v in wlab))
            W = max(W, len(cls))
            cw.append(cls)
        wins.append(cw)
    return order, wins, W


def kernel(x: np.ndarray, labels: np.ndarray, centers: np.ndarray) -> np.ndarray:
    x = np.ascontiguousarray(x, dtype=np.float32)
    labels = np.asarray(labels)
    centers = np.ascontiguousarray(centers, dtype=np.float32)
    nb, d = x.shape
    ncls, k, _ = centers.shape
    assert (nb, d, k) == (B, D, K)

    order, wins, W = _pack_sorted(labels)
    lab_sorted = labels[order]
    wk = W * K
    crows = NPAIR * wk
    crp = ((crows + P - 1) // P) * P

    in_maps = []
    for core in range(NCORES):
        rows = order[core * CORE_ROWS:(core + 1) * CORE_ROWS]
        rl = lab_sorted[core * CORE_ROWS:(core + 1) * CORE_ROWS]
        import ml_dtypes
        xg = np.ascontiguousarray(x[rows]).view(np.uint16)[:, 1::2]
        xg = np.ascontiguousarray(xg).view(ml_dtypes.bfloat16)
        cent = np.zeros((crp, d), dtype=np.float32)
        uts = np.zeros((W, NCH * P + wk), dtype=np.float32)
        for c in range(W):
            uts[c, NCH * P:] = NEG
            uts[c, NCH * P + c * K: NCH * P + (c + 1) * K] = 0.0
        for q in range(NPAIR):
            cls = wins[core][q]
            for c, cl in enumerate(cls):
                cent[q * wk + c * K: q * wk + (c + 1) * K] = centers[cl]
            # per-slot indicators: sub-chunk i=2q+r, slot p = row 2p+r
            wl = rl[q * 2 * P:(q + 1) * 2 * P]
            for r in range(2):
                i = 2 * q + r
                sl = wl[np.arange(P) * 2 + r]        # labels per slot
                for c, cl in enumerate(cls):
                    uts[c, i * P:(i + 1) * P] = (sl == cl).astype(np.float32)
        import ml_dtypes
        in_maps.append(
            {"xg": xg, "cent": cent, "ut": uts.astype(ml_dtypes.bfloat16)}
        )

    nc = build_bass(W)
    res = run_bass_kernel_spmd(nc, in_maps, core_ids=list(range(NCORES)))
    total = sum(float(r["partial"][0, 0]) for r in res.results)
    return np.float32(1.0 - total / nb)
